# revision 2
# baseline (speedup 1.0000x reference)
"""Trainium2 Bass kernel v2 for EnhancedGATModel (3-layer GATv2, N=50000, E=800000).

Strategy (8 NeuronCores, graph-partitioned by destination node):
- Host: append self-loops, sort edges by dst, partition dst nodes 6250/core,
  bucket edges per 128-dst block, split each block's edges by src half
  (int16 gather indices), pad to 128-edge tiles uniformly across cores.
- Device (single SPMD NEFF, all-bf16 tables):
  * per-layer node tables xl = h@Wl (own shard, bf16) + AllGather -> full table
  * per block: one dma_gather of xl[src] + xr[dst] rows into block-wide tiles,
    per-edge math (add/leaky/att-dot) as block-wide stt ops in bf16 (DVE 4x),
    per-head dot via binary-tree adds, exp batched per block,
    scatter via per-head ex-weighted one-hot matmuls accumulating in PSUM
    (denominator via ones-column matmul)
  * BN/relu/residual fused stt per node block; final log_softmax via Softplus.
"""
import sys
import numpy as np

sys.path.insert(0, "/opt/trn_rl_repo")

import ml_dtypes
import concourse.bass as bass
import concourse.mybir as mybir
import concourse.tile as tile
from concourse import bacc
from concourse.bass_utils import run_bass_kernel_spmd

F32 = mybir.dt.float32
BF16 = mybir.dt.bfloat16
I16 = mybir.dt.int16
AF = mybir.ActivationFunctionType
ALU = mybir.AluOpType
BFNP = ml_dtypes.bfloat16

NCORES = 8
BLOCK = 128
D_IN, HID, HEADS, OUT = 128, 64, 4, 2
HC = HEADS * HID  # 256
NEG_SLOPE = 0.2
BN_EPS = 1e-5
GMAX = 8  # dma_gather indices per op = 128*GMAX (HW limit 1024 idx)


# ---------------------------------------------------------------- host prep
def preprocess(edge_index, N):
    """Per-core gather index / dst-local arrays and the tile schedule.

    schedule: list of (block, 'lo'|'hi', ntiles) in tile order; uniform
    across cores. Edge k of a (core,block,half) group lands at partition
    k%128 of tile k//128; pads use src-index 0 (finite reads) and
    dst_local=200 (masked out of the one-hot).
    """
    NPC = N // NCORES
    NBLK = (NPC + BLOCK - 1) // BLOCK
    HALF = N // 2
    src = np.concatenate([edge_index[0], np.arange(N)]).astype(np.int64)
    dst = np.concatenate([edge_index[1], np.arange(N)]).astype(np.int64)
    order = np.argsort(dst, kind="stable")
    src, dst = src[order], dst[order]
    core_of = dst // NPC
    groups = {}
    for c in range(NCORES):
        m = core_of == c
        sc, dc = src[m], dst[m]
        loc = dc - c * NPC
        blk = loc // BLOCK
        lo = sc < HALF
        for b in range(NBLK):
            mb = blk == b
            groups[(c, b, 0)] = (sc[mb & lo], loc[mb & lo] % BLOCK)
            groups[(c, b, 1)] = (sc[mb & ~lo] - HALF, loc[mb & ~lo] % BLOCK)
    schedule = []
    for b in range(NBLK):
        for h, nm in ((0, "lo"), (1, "hi")):
            mx = max(len(groups[(c, b, h)][0]) for c in range(NCORES))
            T = (mx + 127) // 128
            if T > 0:
                schedule.append((b, nm, T))
    TT = sum(T for _, _, T in schedule)
    idx_xl = np.zeros((NCORES, 128, 8 * TT), np.int16)
    idx_xr = np.zeros((NCORES, 128, 8 * TT), np.int16)
    dstl = np.full((NCORES, 128, TT), 200.0, np.float32)
    t0 = 0
    for b, nm, T in schedule:
        h = 0 if nm == "lo" else 1
        for c in range(NCORES):
            s, dl = groups[(c, b, h)]
            ne = len(s)
            pad = T * 128 - ne
            sp = np.concatenate([s, np.zeros(pad, np.int64)]).astype(np.int64)
            dlp = np.concatenate([dl, np.full(pad, 200)]).astype(np.int64)
            wrap = sp.reshape(8 * T, 16).T.astype(np.int16)
            idx_xl[c, :, 8 * t0:8 * (t0 + T)] = np.tile(wrap, (8, 1))
            xr = b * BLOCK + np.minimum(dlp, BLOCK - 1)
            xr = np.minimum(xr, N // NCORES - 1)
            wrap2 = xr.reshape(8 * T, 16).T.astype(np.int16)
            idx_xr[c, :, 8 * t0:8 * (t0 + T)] = np.tile(wrap2, (8, 1))
            dstl[c, :, t0:t0 + T] = dlp.reshape(T, 128).T.astype(np.float32)
        t0 += T
    return idx_xl, idx_xr, dstl, schedule, NBLK


def _mkpack(dtype):
    cols = {}
    parts = []
    c0 = [0]

    def add(name, arr):
        a = np.zeros((128, arr.shape[1]), dtype)
        a[:arr.shape[0]] = arr.astype(np.float32)
        cols[name] = (arr.shape[0], c0[0], arr.shape[1])
        parts.append(a)
        c0[0] += arr.shape[1]

    return cols, parts, add


def pack_consts(ip):
    """Two packed const tensors: f32 and bf16."""
    f = lambda k: np.asarray(ip[k], np.float32)
    bcast = lambda v: np.broadcast_to(
        np.asarray(v, np.float32)[None, :], (128, len(np.asarray(v)))).copy()

    colsF, partsF, addF = _mkpack(np.float32)
    iota = np.broadcast_to(np.arange(128, dtype=np.float32), (128, 128))
    addF("iotaC", np.arange(128, dtype=np.float32)[:, None])
    addF("bias2B", bcast(f("bias2").reshape(-1)))
    addF("b_in", f("b_in").reshape(-1, 1))

    # column permutation: new j -> old h*HID+c with h=j%HEADS, c=j//HEADS
    PERM = np.array([(j % HEADS) * HID + (j // HEADS) for j in range(HC)])
    colsB, partsB, addB = _mkpack(BFNP)
    addB("iota", np.ascontiguousarray(iota))
    addB("one", np.ones((128, 1), np.float32))
    addB("attB0", bcast(f("att0").reshape(-1)[PERM]))
    addB("attB1", bcast(f("att1").reshape(-1)[PERM]))
    addB("attB2", bcast(f("att2").reshape(-1)))
    g, bt = f("bn_gamma"), f("bn_beta")
    mu, var = f("bn_mean"), f("bn_var")
    for l in range(2):
        a = g[l] / np.sqrt(var[l] + BN_EPS)
        b = bt[l] - mu[l] * a + a * f(f"bias{l}")
        addB(f"aB{l}", bcast(a[PERM]))
        addB(f"bB{l}", bcast(b[PERM]))
    addB("W_in", f("W_in"))
    addB("Wl0", f("Wl0")[:, PERM])
    addB("Wr0", f("Wr0")[:, PERM])
    Wl1, Wr1 = f("Wl1")[PERM][:, PERM], f("Wr1")[PERM][:, PERM]
    addB("Wl1k0", Wl1[:128]); addB("Wl1k1", Wl1[128:])
    addB("Wr1k0", Wr1[:128]); addB("Wr1k1", Wr1[128:])
    Wl2, Wr2 = f("Wl2")[PERM], f("Wr2")[PERM]
    addB("Wl2k0", Wl2[:128]); addB("Wl2k1", Wl2[128:])
    addB("Wr2k0", Wr2[:128]); addB("Wr2k1", Wr2[128:])
    cF = np.concatenate(partsF, axis=1)
    cB = np.concatenate(partsB, axis=1)
    return cF, colsF, cB, colsB


def _gather(nc, out_tile, in_ap, idx_tile, tstart, tout, T, elem):
    """Chunked dma_gather: out_tile[:, tout+k, :] = table[idx[tile tstart+k]]."""
    k = 0
    while k < T:
        Tc = min(GMAX, T - k)
        nc.gpsimd.dma_gather(
            out_ap=out_tile[:, tout + k:tout + k + Tc, :], in_ap=in_ap,
            idxs_ap=idx_tile[:, 8 * (tstart + k):8 * (tstart + k + Tc)],
            num_idxs=128 * Tc, num_idxs_reg=128 * Tc, elem_size=elem)
        k += Tc


def _chunks(NPC):
    out = []
    st = 0
    while st < NPC:
        out.append((st, min(128, NPC - st)))
        st += 128
    return out


def build(N, schedule, NBLK, TT, CWF, CWB):
    NPC = N // NCORES
    HALF = N // 2
    nc = bacc.Bacc("TRN2", target_bir_lowering=False, debug=False)

    xT = nc.dram_tensor("xT", [D_IN, NPC], BF16, kind="ExternalInput")
    idx_xl = nc.dram_tensor("idx_xl", [128, 8 * TT], I16, kind="ExternalInput")
    idx_xr = nc.dram_tensor("idx_xr", [128, 8 * TT], I16, kind="ExternalInput")
    dstl = nc.dram_tensor("dstl", [128, TT], F32, kind="ExternalInput")
    constsF = nc.dram_tensor("constsF", [128, CWF], F32, kind="ExternalInput")
    constsB = nc.dram_tensor("constsB", [128, CWB], BF16, kind="ExternalInput")
    out = nc.dram_tensor("out", [NPC, OUT], F32, kind="ExternalOutput")

    xl0_own = nc.dram_tensor("xl0_own", [NPC, HC], BF16)
    xl0_full = nc.dram_tensor("xl0_full", [N, HC], BF16, addr_space="Shared")
    xr0 = nc.dram_tensor("xr0", [NPC, HC], BF16)
    h1_own = nc.dram_tensor("h1_own", [NPC, HC], BF16)
    h1T = nc.dram_tensor("h1T", [HC, NPC], BF16)
    xl1_own = nc.dram_tensor("xl1_own", [NPC, HC], BF16)
    xl1_full = nc.dram_tensor("xl1_full", [N, HC], BF16, addr_space="Shared")
    xr1 = nc.dram_tensor("xr1", [NPC, HC], BF16)
    h2T = nc.dram_tensor("h2T", [HC, NPC], BF16)
    xl2p8_own = nc.dram_tensor("xl2p8_own", [NPC, 8], BF16)
    xl2p8_full = nc.dram_tensor("xl2p8_full", [N, 8], BF16, addr_space="Shared")
    xl2pB = nc.dram_tensor("xl2pB", [N, 128], BF16)  # cols 0:8 valid
    xr2p = nc.dram_tensor("xr2p", [NPC, 128], BF16)  # cols 0:2 valid

    chunks = _chunks(NPC)
    blk_tiles = {b: [] for b in range(NBLK)}
    t0 = 0
    for b, nm, T in schedule:
        blk_tiles[b].append((t0, nm, T))
        t0 += T
    TMAX = max(sum(T for _, _, T in blk_tiles[b]) for b in range(NBLK))

    rg = [list(range(NCORES))]

    with tile.TileContext(nc) as tc:
        import contextlib
        with contextlib.ExitStack() as ctx:
            cst = ctx.enter_context(tc.tile_pool(name="cst", bufs=1))
            sb = ctx.enter_context(tc.tile_pool(name="sb", bufs=3))
            eb = ctx.enter_context(tc.tile_pool(name="eb", bufs=2))
            eb1 = ctx.enter_context(tc.tile_pool(name="eb1", bufs=1))
            gat = ctx.enter_context(tc.tile_pool(name="gat", bufs=2))
            ohp = ctx.enter_context(tc.tile_pool(name="ohp", bufs=6))
            ps = ctx.enter_context(tc.tile_pool(name="ps", bufs=2, space="PSUM"))
            psa = ctx.enter_context(tc.tile_pool(name="psa", bufs=2, space="PSUM"))

            CF = cst.tile([128, CWF], F32)
            nc.sync.dma_start(CF[:], constsF[:])
            CB = cst.tile([128, CWB], BF16)
            nc.sync.dma_start(CB[:], constsB[:])

            def csF(name):
                r, c0i, w = COLSF[name]
                return CF[0:r, c0i:c0i + w]

            def csB(name):
                r, c0i, w = COLSB[name]
                return CB[0:r, c0i:c0i + w]

            identB = cst.tile([128, 128], BF16)
            nc.vector.tensor_scalar(out=identB[:], in0=csB("iota"),
                                    scalar1=csF("iotaC"), scalar2=None,
                                    op0=ALU.is_equal)
            ixl_t = cst.tile([128, 8 * TT], I16)
            nc.sync.dma_start(ixl_t[:], idx_xl[:])
            ixr_t = cst.tile([128, 8 * TT], I16)
            nc.sync.dma_start(ixr_t[:], idx_xr[:])
            dstl_t = cst.tile([128, TT], F32)
            nc.sync.dma_start(dstl_t[:], dstl[:])
            o_all = cst.tile([128, 2 * NBLK], F32)
            h0T = cst.tile([64, NPC], BF16)

            # ---------------- phase A: L0 node prep ----------------
            for st, sz in chunks:
                xTc = sb.tile([D_IN, 128], BF16, tag="xTc")
                nc.sync.dma_start(xTc[:, :sz], xT[:, st:st + sz])
                p1 = psa.tile([64, 128], F32, tag="prep", space="PSUM")
                nc.tensor.matmul(p1[:, :sz], lhsT=csB("W_in"), rhs=xTc[:, :sz],
                                 start=True, stop=True)
                nc.scalar.activation(h0T[:, st:st + sz], p1[:, :sz], AF.Relu,
                                     bias=csF("b_in"))
                for W, tab in (("Wl0", xl0_own), ("Wr0", xr0)):
                    p2 = psa.tile([128, HC], F32, tag="prep", space="PSUM")
                    nc.tensor.matmul(p2[:sz, :], lhsT=h0T[:, st:st + sz],
                                     rhs=csB(W), start=True, stop=True)
                    cp = sb.tile([128, HC], BF16, tag="cpA")
                    nc.scalar.copy(cp[:sz, :], p2[:sz, :])
                    nc.sync.dma_start(tab[st:st + sz, :], cp[:sz, :])

            nc.gpsimd.collective_compute(
                "AllGather", ALU.bypass, ins=[xl0_own[:]], outs=[xl0_full[:]],
                replica_groups=rg)

            # ---------------- edge pass for layers 0/1 ----------------
            def edge_pass(lidx, xl_full, xr_tab, attB, aB, bB, hT_out,
                          h_own_out, residual):
                for b in range(NBLK):
                    st = b * BLOCK
                    nreal = min(BLOCK, NPC - st)
                    tl = blk_tiles[b]
                    T_all = sum(T for _, _, T in tl)
                    tglob0 = tl[0][0]
                    # gathers into one block-wide tile
                    gxl = gat.tile([128, TMAX, HC], BF16, tag="gxl")
                    tloc = 0
                    for (tg, nm, T) in tl:
                        src_ap = xl_full[0:HALF, :] if nm == "lo" else xl_full[HALF:N, :]
                        _gather(nc, gxl, src_ap, ixl_t, tg, tloc, T, HC)
                        tloc += T
                    gxr = gat.tile([128, TMAX, HC], BF16, tag="gxr")
                    _gather(nc, gxr, xr_tab[:], ixr_t, tglob0, 0, T_all, HC)
                    if residual is not None:
                        hres = sb.tile([128, HC], BF16, tag="hres")
                        if nreal < 128:
                            nc.vector.memset(hres[:], 0.0)
                        nc.sync.dma_start(hres[:nreal, :], residual[st:st + nreal, :])
                    # block-wide edge math (TT 2x / TS 4x ops)
                    u = eb1.tile([128, TMAX, HC], BF16, tag="u")
                    nc.vector.tensor_tensor(
                        out=u[:, :T_all, :], in0=gxl[:, :T_all, :],
                        in1=gxr[:, :T_all, :], op=ALU.add)
                    m = eb1.tile([128, TMAX, HC], BF16, tag="m")
                    nc.scalar.activation(m[:, :T_all, :], u[:, :T_all, :],
                                         AF.Prelu, alpha=NEG_SLOPE)
                    p = eb1.tile([128, TMAX, HID, HEADS], BF16, tag="p")
                    nc.vector.tensor_tensor(
                        out=p[:, :T_all, :, :].rearrange("a t c h -> a t (c h)"),
                        in0=m[:, :T_all, :],
                        in1=attB[:, None, :].to_broadcast([128, T_all, HC]),
                        op=ALU.mult)
                    # binary-tree per-head reduction (TT 2x, heads innermost)
                    q1 = eb1.tile([128, TMAX, 32, HEADS], BF16, tag="q1")
                    nc.vector.tensor_tensor(
                        out=q1[:, :T_all], in0=p[:, :T_all, 0:32, :],
                        in1=p[:, :T_all, 32:64, :], op=ALU.add)
                    q2 = eb1.tile([128, TMAX, 16, HEADS], BF16, tag="q2")
                    nc.vector.tensor_tensor(
                        out=q2[:, :T_all], in0=q1[:, :T_all, 0:16, :],
                        in1=q1[:, :T_all, 16:32, :], op=ALU.add)
                    q3 = eb1.tile([128, TMAX, 8, HEADS], BF16, tag="q3")
                    nc.vector.tensor_tensor(
                        out=q3[:, :T_all], in0=q2[:, :T_all, 0:8, :],
                        in1=q2[:, :T_all, 8:16, :], op=ALU.add)
                    q4 = eb1.tile([128, TMAX, 4, HEADS], BF16, tag="q4")
                    nc.vector.tensor_tensor(
                        out=q4[:, :T_all], in0=q3[:, :T_all, 0:4, :],
                        in1=q3[:, :T_all, 4:8, :], op=ALU.add)
                    q5 = eb1.tile([128, TMAX, 2, HEADS], BF16, tag="q5")
                    nc.vector.tensor_tensor(
                        out=q5[:, :T_all], in0=q4[:, :T_all, 0:2, :],
                        in1=q4[:, :T_all, 2:4, :], op=ALU.add)
                    lg = eb1.tile([128, TMAX, HEADS], F32, tag="lg")
                    nc.vector.tensor_tensor(
                        out=lg[:, :T_all], in0=q5[:, :T_all, 0, :],
                        in1=q5[:, :T_all, 1, :], op=ALU.add)
                    # combined scatter rhs: [ex-weighted xl | ex] per edge
                    rhsC = eb.tile([128, TMAX, HC + HEADS], BF16, tag="rhsC")
                    ex = rhsC[:, :, HC:HC + HEADS]
                    nc.scalar.activation(ex[:, :T_all], lg[:, :T_all], AF.Exp)
                    # block-wide ex premultiply (heads innermost -> packed 2x)
                    nc.vector.tensor_tensor(
                        out=rhsC[:, :T_all, 0:HC].rearrange("a t (c h) -> a t c h", h=HEADS),
                        in0=gxl[:, :T_all, :].rearrange("a t (c h) -> a t c h", h=HEADS),
                        in1=ex[:, :T_all, None, :].to_broadcast(
                            [128, T_all, HID, HEADS]),
                        op=ALU.mult)
                    # scatter: one matmul per tile (numerator + denominator)
                    acc = ps.tile([128, HC + HEADS], F32, tag="acc", space="PSUM")
                    for t in range(T_all):
                        gt = tglob0 + t
                        oh = ohp.tile([128, 128], BF16, tag="oh")
                        nc.vector.tensor_scalar(
                            out=oh[:], in0=csB("iota"),
                            scalar1=dstl_t[:, gt:gt + 1], scalar2=None,
                            op0=ALU.is_equal)
                        nc.tensor.matmul(
                            acc[:], lhsT=oh[:], rhs=rhsC[:, t, :],
                            start=(t == 0), stop=(t == T_all - 1))
                    # block post
                    rc = sb.tile([128, HEADS], F32, tag="rc")
                    nc.vector.reciprocal(rc[:], acc[:, HC:HC + HEADS])
                    go = sb.tile([128, HID, HEADS], BF16, tag="go")
                    nc.vector.tensor_tensor(
                        out=go[:], in0=acc[:, 0:HC].rearrange(
                            "a (c h) -> a c h", h=HEADS),
                        in1=rc[:, None, :].to_broadcast([128, HID, HEADS]),
                        op=ALU.mult)
                    t1 = sb.tile([128, HC], BF16, tag="t1")
                    nc.vector.tensor_tensor(
                        out=t1[:], in0=go[:].rearrange("a c h -> a (c h)"),
                        in1=aB, op=ALU.mult)
                    t2 = sb.tile([128, HC], BF16, tag="t2")
                    nc.vector.tensor_tensor(
                        out=t2[:], in0=t1[:], in1=bB, op=ALU.add)
                    h_t = sb.tile([128, HC], BF16, tag="h")
                    if residual is not None:
                        r_t = sb.tile([128, HC], BF16, tag="r")
                        nc.vector.tensor_scalar(
                            out=r_t[:], in0=t2[:], scalar1=0.0, scalar2=None,
                            op0=ALU.max)
                        nc.vector.tensor_tensor(
                            out=h_t[:], in0=r_t[:], in1=hres[:], op=ALU.add)
                    else:
                        nc.vector.tensor_scalar(
                            out=h_t[:], in0=t2[:], scalar1=0.0, scalar2=None,
                            op0=ALU.max)
                    if h_own_out is not None:
                        nc.sync.dma_start(h_own_out[st:st + nreal, :], h_t[:nreal, :])
                    for half in range(2):
                        tp = ps.tile([128, 128], BF16, tag="tp", space="PSUM")
                        nc.tensor.transpose(tp[:], h_t[:, half * 128:(half + 1) * 128],
                                            identB[:])
                        tcp = sb.tile([128, 128], BF16, tag="tcp")
                        nc.vector.tensor_copy(tcp[:], tp[:])
                        nc.sync.dma_start(hT_out[half * 128:(half + 1) * 128,
                                                 st:st + nreal], tcp[:, :nreal])

            edge_pass(0, xl0_full, xr0, csB("attB0"), csB("aB0"), csB("bB0"),
                      h1T, h1_own, None)

            # ---------------- phase C: L1 node prep ----------------
            for st, sz in chunks:
                ht0 = sb.tile([128, 128], BF16, tag="ht0")
                nc.sync.dma_start(ht0[:, :sz], h1T[0:128, st:st + sz])
                ht1 = sb.tile([128, 128], BF16, tag="ht1")
                nc.sync.dma_start(ht1[:, :sz], h1T[128:256, st:st + sz])
                for Wk0, Wk1, tab in (("Wl1k0", "Wl1k1", xl1_own),
                                      ("Wr1k0", "Wr1k1", xr1)):
                    p2 = psa.tile([128, HC], F32, tag="prep", space="PSUM")
                    nc.tensor.matmul(p2[:sz, :], lhsT=ht0[:, :sz], rhs=csB(Wk0),
                                     start=True, stop=False)
                    nc.tensor.matmul(p2[:sz, :], lhsT=ht1[:, :sz], rhs=csB(Wk1),
                                     start=False, stop=True)
                    cp = sb.tile([128, HC], BF16, tag="cpA")
                    nc.scalar.copy(cp[:sz, :], p2[:sz, :])
                    nc.sync.dma_start(tab[st:st + sz, :], cp[:sz, :])

            nc.gpsimd.collective_compute(
                "AllGather", ALU.bypass, ins=[xl1_own[:]], outs=[xl1_full[:]],
                replica_groups=rg)

            edge_pass(1, xl1_full, xr1, csB("attB1"), csB("aB1"), csB("bB1"),
                      h2T, None, h1_own)

            # ---------------- phase E: L2 node prep ----------------
            for st, sz in chunks:
                ht0 = sb.tile([128, 128], BF16, tag="ht0")
                nc.sync.dma_start(ht0[:, :sz], h2T[0:128, st:st + sz])
                ht1 = sb.tile([128, 128], BF16, tag="ht1")
                nc.sync.dma_start(ht1[:, :sz], h2T[128:256, st:st + sz])
                p2 = psa.tile([128, OUT], F32, tag="prep", space="PSUM")
                nc.tensor.matmul(p2[:sz, :], lhsT=ht0[:, :sz], rhs=csB("Wl2k0"),
                                 start=True, stop=False)
                nc.tensor.matmul(p2[:sz, :], lhsT=ht1[:, :sz], rhs=csB("Wl2k1"),
                                 start=False, stop=True)
                cp8 = sb.tile([128, 8], BF16, tag="cp8")
                nc.vector.memset(cp8[:], 0.0)
                nc.scalar.copy(cp8[:sz, 0:OUT], p2[:sz, :])
                nc.sync.dma_start(xl2p8_own[st:st + sz, :], cp8[:sz, :])
                p3 = psa.tile([128, OUT], F32, tag="prep", space="PSUM")
                nc.tensor.matmul(p3[:sz, :], lhsT=ht0[:, :sz], rhs=csB("Wr2k0"),
                                 start=True, stop=False)
                nc.tensor.matmul(p3[:sz, :], lhsT=ht1[:, :sz], rhs=csB("Wr2k1"),
                                 start=False, stop=True)
                cpr = sb.tile([128, 128], BF16, tag="cpr")
                nc.vector.memset(cpr[:], 0.0)
                nc.scalar.copy(cpr[:sz, 0:OUT], p3[:sz, :])
                nc.sync.dma_start(xr2p[st:st + sz, :], cpr[:sz, :])

            nc.gpsimd.collective_compute(
                "AllGather", ALU.bypass, ins=[xl2p8_own[:]], outs=[xl2p8_full[:]],
                replica_groups=rg)
            # expand [N, 8] -> cols 0:8 of [N, 128] (cols 8: stay garbage, unused)
            nc.sync.dma_start(xl2pB[:, 0:8], xl2p8_full[:])

            # ---------------- phase F: L2 edge pass ----------------
            att2 = csB("attB2")
            for b in range(NBLK):
                st = b * BLOCK
                nreal = min(BLOCK, NPC - st)
                tl = blk_tiles[b]
                T_all = sum(T for _, _, T in tl)
                tglob0 = tl[0][0]
                gxl = gat.tile([128, TMAX, 128], BF16, tag="gxl")
                tloc = 0
                for (tg, nm, T) in tl:
                    src_ap = xl2pB[0:HALF, :] if nm == "lo" else xl2pB[HALF:N, :]
                    _gather(nc, gxl, src_ap, ixl_t, tg, tloc, T, 128)
                    tloc += T
                gxr = gat.tile([128, TMAX, 128], BF16, tag="gxr")
                _gather(nc, gxr, xr2p[:], ixr_t, tglob0, 0, T_all, 128)
                u2 = eb1.tile([128, TMAX, OUT], BF16, tag="u2")
                nc.vector.scalar_tensor_tensor(
                    out=u2[:, :T_all, :], in0=gxl[:, :T_all, 0:OUT], scalar=1.0,
                    in1=gxr[:, :T_all, 0:OUT], op0=ALU.mult, op1=ALU.add)
                m2 = eb1.tile([128, TMAX, OUT], BF16, tag="m2")
                nc.vector.scalar_tensor_tensor(
                    out=m2[:, :T_all, :], in0=u2[:, :T_all, :], scalar=NEG_SLOPE,
                    in1=u2[:, :T_all, :], op0=ALU.mult, op1=ALU.max)
                p2_ = eb1.tile([128, TMAX, OUT], BF16, tag="p2")
                nc.vector.scalar_tensor_tensor(
                    out=p2_[:, :T_all, :], in0=m2[:, :T_all, :], scalar=1.0,
                    in1=att2[:, None, :].to_broadcast([128, T_all, OUT]),
                    op0=ALU.mult, op1=ALU.mult)
                lg2 = eb1.tile([128, TMAX], F32, tag="lg2")
                nc.vector.scalar_tensor_tensor(
                    out=lg2[:, :T_all], in0=p2_[:, :T_all, 0], scalar=1.0,
                    in1=p2_[:, :T_all, 1], op0=ALU.mult, op1=ALU.add)
                rhsC2 = eb.tile([128, TMAX, OUT + 1], BF16, tag="rhsC2")
                ex2 = rhsC2[:, :, OUT:OUT + 1]
                nc.scalar.activation(ex2[:, :T_all, 0], lg2[:, :T_all], AF.Exp)
                nc.vector.tensor_tensor(
                    out=rhsC2[:, :T_all, 0:OUT], in0=gxl[:, :T_all, 0:OUT],
                    in1=ex2[:, :T_all, :].to_broadcast([128, T_all, OUT]),
                    op=ALU.mult)
                acc2 = ps.tile([128, OUT + 1], F32, tag="acc", space="PSUM")
                for t in range(T_all):
                    gt = tglob0 + t
                    oh = ohp.tile([128, 128], BF16, tag="oh")
                    nc.vector.tensor_scalar(
                        out=oh[:], in0=csB("iota"),
                        scalar1=dstl_t[:, gt:gt + 1], scalar2=None,
                        op0=ALU.is_equal)
                    nc.tensor.matmul(acc2[:], lhsT=oh[:], rhs=rhsC2[:, t, :],
                                     start=(t == 0), stop=(t == T_all - 1))
                rc2 = sb.tile([128, 1], F32, tag="rc2")
                nc.vector.reciprocal(rc2[:], acc2[:, OUT:OUT + 1])
                o2 = sb.tile([128, OUT], F32, tag="o2")
                nc.vector.tensor_scalar(out=o2[:], in0=acc2[:, 0:OUT],
                                        scalar1=rc2[:], scalar2=None,
                                        op0=ALU.mult)
                nc.vector.tensor_tensor(out=o_all[:, 2 * b:2 * b + 2], in0=o2[:],
                                        in1=csF("bias2B"), op=ALU.add)

            # ---------------- phase G: log_softmax ----------------
            for b in range(NBLK):
                st = b * BLOCK
                nreal = min(BLOCK, NPC - st)
                d = sb.tile([128, 1], F32, tag="d")
                nc.vector.tensor_tensor(out=d[:], in0=o_all[:, 2 * b + 1:2 * b + 2],
                                        in1=o_all[:, 2 * b:2 * b + 1],
                                        op=ALU.subtract)
                e = sb.tile([128, 1], F32, tag="e")
                nc.scalar.activation(e[:], d[:], AF.Exp)
                ep1 = sb.tile([128, 1], F32, tag="ep1")
                nc.vector.tensor_scalar(out=ep1[:], in0=e[:], scalar1=1.0,
                                        scalar2=None, op0=ALU.add)
                l = sb.tile([128, 1], F32, tag="l")
                nc.scalar.activation(l[:], ep1[:], AF.Ln)
                ls = sb.tile([128, 2], F32, tag="ls")
                nc.vector.tensor_scalar(out=ls[:, 0:1], in0=l[:], scalar1=-1.0,
                                        scalar2=None, op0=ALU.mult)
                nc.vector.tensor_tensor(out=ls[:, 1:2], in0=d[:], in1=l[:],
                                        op=ALU.subtract)
                nc.sync.dma_start(out[st:st + nreal, :], ls[:nreal, :])

    nc.compile()
    return nc


COLSF = None
COLSB = None


# ---------------------------------------------------------------- entry
_CACHE = {}
LAST_RESULTS = None
LAST_NC = None


def kernel(**inputs):
    global COLSF, COLSB, LAST_RESULTS, LAST_NC
    x = np.asarray(inputs["x"], np.float32)
    ei = np.asarray(inputs["edge_index"]).astype(np.int64)
    N = x.shape[0]
    NPC = N // NCORES

    idx_xl, idx_xr, dstl, schedule, NBLK = preprocess(ei, N)
    TT = sum(T for _, _, T in schedule)
    cF, COLSF, cB, COLSB = pack_consts(inputs)

    key = (N, TT, NBLK, tuple(schedule))
    if key not in _CACHE:
        _CACHE[key] = build(N, schedule, NBLK, TT, cF.shape[1], cB.shape[1])
    nc = _CACHE[key]
    LAST_NC = nc

    in_maps = []
    for c in range(NCORES):
        sl = slice(c * NPC, (c + 1) * NPC)
        in_maps.append(dict(
            xT=np.ascontiguousarray(x[sl].T).astype(BFNP),
            idx_xl=idx_xl[c], idx_xr=idx_xr[c], dstl=dstl[c],
            constsF=cF, constsB=cB,
        ))
    res = run_bass_kernel_spmd(nc, in_maps, list(range(NCORES)))
    LAST_RESULTS = res
    outs = [res.results[c]["out"] for c in range(NCORES)]
    return np.concatenate(outs, axis=0).astype(np.float32)


# revision 6
# speedup vs baseline: 1.0470x; 1.0470x over previous
"""Trainium2 Bass kernel v2 for EnhancedGATModel (3-layer GATv2, N=50000, E=800000).

Strategy (8 NeuronCores, graph-partitioned by destination node):
- Host: append self-loops, sort edges by dst, partition dst nodes 6250/core,
  bucket edges per 128-dst block, split each block's edges by src half
  (int16 gather indices), pad to 128-edge tiles uniformly across cores.
- Device (single SPMD NEFF, all-bf16 tables):
  * per-layer node tables xl = h@Wl (own shard, bf16) + AllGather -> full table
  * per block: one dma_gather of xl[src] + xr[dst] rows into block-wide tiles,
    per-edge math (add/leaky/att-dot) as block-wide stt ops in bf16 (DVE 4x),
    per-head dot via binary-tree adds, exp batched per block,
    scatter via per-head ex-weighted one-hot matmuls accumulating in PSUM
    (denominator via ones-column matmul)
  * BN/relu/residual fused stt per node block; final log_softmax via Softplus.
"""
import sys
import numpy as np

sys.path.insert(0, "/opt/trn_rl_repo")

import ml_dtypes
import concourse.bass as bass
import concourse.mybir as mybir
import concourse.tile as tile
from concourse import bacc
from concourse.bass_utils import run_bass_kernel_spmd

F32 = mybir.dt.float32
BF16 = mybir.dt.bfloat16
I16 = mybir.dt.int16
AF = mybir.ActivationFunctionType
ALU = mybir.AluOpType
BFNP = ml_dtypes.bfloat16

NCORES = 8
BLOCK = 128
D_IN, HID, HEADS, OUT = 128, 64, 4, 2
HC = HEADS * HID  # 256
NEG_SLOPE = 0.2
BN_EPS = 1e-5
GMAX = 8  # dma_gather indices per op = 128*GMAX (HW limit 1024 idx)


# ---------------------------------------------------------------- host prep
def preprocess(edge_index, N):
    """Per-core gather index / dst-local arrays and the tile schedule.

    schedule: list of (block, 'lo'|'hi', ntiles) in tile order; uniform
    across cores. Edge k of a (core,block,half) group lands at partition
    k%128 of tile k//128; pads use src-index 0 (finite reads) and
    dst_local=200 (masked out of the one-hot).
    """
    NPC = N // NCORES
    NBLK = (NPC + BLOCK - 1) // BLOCK
    HALF = N // 2
    src = np.concatenate([edge_index[0], np.arange(N)]).astype(np.int64)
    dst = np.concatenate([edge_index[1], np.arange(N)]).astype(np.int64)
    order = np.argsort(dst, kind="stable")
    src, dst = src[order], dst[order]
    core_of = dst // NPC
    groups = {}
    for c in range(NCORES):
        m = core_of == c
        sc, dc = src[m], dst[m]
        loc = dc - c * NPC
        blk = loc // BLOCK
        lo = sc < HALF
        for b in range(NBLK):
            mb = blk == b
            groups[(c, b, 0)] = (sc[mb & lo], loc[mb & lo] % BLOCK)
            groups[(c, b, 1)] = (sc[mb & ~lo] - HALF, loc[mb & ~lo] % BLOCK)
    schedule = []
    for b in range(NBLK):
        for h, nm in ((0, "lo"), (1, "hi")):
            mx = max(len(groups[(c, b, h)][0]) for c in range(NCORES))
            T = (mx + 127) // 128
            if T > 0:
                schedule.append((b, nm, T))
    TT = sum(T for _, _, T in schedule)
    idx_xl = np.zeros((NCORES, 128, 8 * TT), np.int16)
    idx_xr = np.zeros((NCORES, 128, 8 * TT), np.int16)
    dstl = np.full((NCORES, 128, TT), 200.0, np.float32)
    t0 = 0
    for b, nm, T in schedule:
        h = 0 if nm == "lo" else 1
        for c in range(NCORES):
            s, dl = groups[(c, b, h)]
            ne = len(s)
            pad = T * 128 - ne
            sp = np.concatenate([s, np.zeros(pad, np.int64)]).astype(np.int64)
            dlp = np.concatenate([dl, np.full(pad, 200)]).astype(np.int64)
            wrap = sp.reshape(8 * T, 16).T.astype(np.int16)
            idx_xl[c, :, 8 * t0:8 * (t0 + T)] = np.tile(wrap, (8, 1))
            xr = b * BLOCK + np.minimum(dlp, BLOCK - 1)
            xr = np.minimum(xr, N // NCORES - 1)
            wrap2 = xr.reshape(8 * T, 16).T.astype(np.int16)
            idx_xr[c, :, 8 * t0:8 * (t0 + T)] = np.tile(wrap2, (8, 1))
            dstl[c, :, t0:t0 + T] = dlp.reshape(T, 128).T.astype(np.float32)
        t0 += T
    return idx_xl, idx_xr, dstl, schedule, NBLK


def _mkpack(dtype):
    cols = {}
    parts = []
    c0 = [0]

    def add(name, arr):
        a = np.zeros((128, arr.shape[1]), dtype)
        a[:arr.shape[0]] = arr.astype(np.float32)
        cols[name] = (arr.shape[0], c0[0], arr.shape[1])
        parts.append(a)
        c0[0] += arr.shape[1]

    return cols, parts, add


def pack_consts(ip):
    """Two packed const tensors: f32 and bf16."""
    f = lambda k: np.asarray(ip[k], np.float32)
    bcast = lambda v: np.broadcast_to(
        np.asarray(v, np.float32)[None, :], (128, len(np.asarray(v)))).copy()

    colsF, partsF, addF = _mkpack(np.float32)
    iota = np.broadcast_to(np.arange(128, dtype=np.float32), (128, 128))
    addF("iotaC", np.arange(128, dtype=np.float32)[:, None])
    addF("bias2B", bcast(f("bias2").reshape(-1)))
    addF("b_in", f("b_in").reshape(-1, 1))

    # column permutation: new j -> old h*HID+c with h=j%HEADS, c=j//HEADS
    PERM = np.array([(j % HEADS) * HID + (j // HEADS) for j in range(HC)])
    colsB, partsB, addB = _mkpack(BFNP)
    addB("iota", np.ascontiguousarray(iota))
    addB("one", np.ones((128, 1), np.float32))
    addB("attB0", bcast(f("att0").reshape(-1)[PERM]))
    addB("attB1", bcast(f("att1").reshape(-1)[PERM]))
    addB("attB2", bcast(f("att2").reshape(-1)))
    g, bt = f("bn_gamma"), f("bn_beta")
    mu, var = f("bn_mean"), f("bn_var")
    for l in range(2):
        a = g[l] / np.sqrt(var[l] + BN_EPS)
        b = bt[l] - mu[l] * a + a * f(f"bias{l}")
        addB(f"aB{l}", bcast(a[PERM]))
        addB(f"bB{l}", bcast(b[PERM]))
    addB("W_in", f("W_in"))
    addB("Wl0", f("Wl0")[:, PERM])
    addB("Wr0", f("Wr0")[:, PERM])
    Wl1, Wr1 = f("Wl1")[PERM][:, PERM], f("Wr1")[PERM][:, PERM]
    addB("Wl1k0", Wl1[:128]); addB("Wl1k1", Wl1[128:])
    addB("Wr1k0", Wr1[:128]); addB("Wr1k1", Wr1[128:])
    Wl2, Wr2 = f("Wl2")[PERM], f("Wr2")[PERM]
    addB("Wl2k0", Wl2[:128]); addB("Wl2k1", Wl2[128:])
    addB("Wr2k0", Wr2[:128]); addB("Wr2k1", Wr2[128:])
    cF = np.concatenate(partsF, axis=1)
    cB = np.concatenate(partsB, axis=1)
    return cF, colsF, cB, colsB


def _gather(nc, out_tile, in_ap, idx_tile, tstart, tout, T, elem):
    """Chunked dma_gather: out_tile[:, tout+k, :] = table[idx[tile tstart+k]]."""
    k = 0
    while k < T:
        Tc = min(GMAX, T - k)
        nc.gpsimd.dma_gather(
            out_ap=out_tile[:, tout + k:tout + k + Tc, :], in_ap=in_ap,
            idxs_ap=idx_tile[:, 8 * (tstart + k):8 * (tstart + k + Tc)],
            num_idxs=128 * Tc, num_idxs_reg=128 * Tc, elem_size=elem)
        k += Tc


def _chunks(NPC):
    out = []
    st = 0
    while st < NPC:
        out.append((st, min(128, NPC - st)))
        st += 128
    return out


def build(N, schedule, NBLK, TT, CWF, CWB):
    NPC = N // NCORES
    HALF = N // 2
    nc = bacc.Bacc("TRN2", target_bir_lowering=False, debug=False)

    xT = nc.dram_tensor("xT", [D_IN, NPC], BF16, kind="ExternalInput")
    idx_xl = nc.dram_tensor("idx_xl", [128, 8 * TT], I16, kind="ExternalInput")
    idx_xr = nc.dram_tensor("idx_xr", [128, 8 * TT], I16, kind="ExternalInput")
    dstl = nc.dram_tensor("dstl", [128, TT], F32, kind="ExternalInput")
    constsF = nc.dram_tensor("constsF", [128, CWF], F32, kind="ExternalInput")
    constsB = nc.dram_tensor("constsB", [128, CWB], BF16, kind="ExternalInput")
    out = nc.dram_tensor("out", [NPC, OUT], F32, kind="ExternalOutput")

    ohs_d = nc.dram_tensor("ohs_d", [128, TT * 128], BF16)
    xl0_own = nc.dram_tensor("xl0_own", [NPC, HC], BF16)
    xl0_full = nc.dram_tensor("xl0_full", [N, HC], BF16, addr_space="Shared")
    xr0 = nc.dram_tensor("xr0", [NPC, HC], BF16)
    h1_own = nc.dram_tensor("h1_own", [NPC, HC], BF16)
    h1T = nc.dram_tensor("h1T", [HC, NPC], BF16)
    xl1_own = nc.dram_tensor("xl1_own", [NPC, HC], BF16)
    xl1_full = nc.dram_tensor("xl1_full", [N, HC], BF16, addr_space="Shared")
    xr1 = nc.dram_tensor("xr1", [NPC, HC], BF16)
    h2T = nc.dram_tensor("h2T", [HC, NPC], BF16)
    xl2p8_own = nc.dram_tensor("xl2p8_own", [NPC, 8], BF16)
    xl2p8_full = nc.dram_tensor("xl2p8_full", [N, 8], BF16, addr_space="Shared")
    xl2pB = nc.dram_tensor("xl2pB", [N, 128], BF16)  # cols 0:8 valid
    xr2p = nc.dram_tensor("xr2p", [NPC, 128], BF16)  # cols 0:2 valid

    chunks = _chunks(NPC)
    blk_tiles = {b: [] for b in range(NBLK)}
    t0 = 0
    for b, nm, T in schedule:
        blk_tiles[b].append((t0, nm, T))
        t0 += T
    TMAX = max(sum(T for _, _, T in blk_tiles[b]) for b in range(NBLK))

    rg = [list(range(NCORES))]

    with tile.TileContext(nc) as tc:
        import contextlib
        with contextlib.ExitStack() as ctx:
            cst = ctx.enter_context(tc.tile_pool(name="cst", bufs=1))
            sb = ctx.enter_context(tc.tile_pool(name="sb", bufs=3))
            eb = ctx.enter_context(tc.tile_pool(name="eb", bufs=2))
            eb1 = ctx.enter_context(tc.tile_pool(name="eb1", bufs=1))
            gat = ctx.enter_context(tc.tile_pool(name="gat", bufs=2))
            ohp = ctx.enter_context(tc.tile_pool(name="ohp", bufs=2))
            ps = ctx.enter_context(tc.tile_pool(name="ps", bufs=2, space="PSUM"))
            psa = ctx.enter_context(tc.tile_pool(name="psa", bufs=2, space="PSUM"))

            CF = cst.tile([128, CWF], F32)
            nc.sync.dma_start(CF[:], constsF[:])
            CB = cst.tile([128, CWB], BF16)
            nc.sync.dma_start(CB[:], constsB[:])

            def csF(name):
                r, c0i, w = COLSF[name]
                return CF[0:r, c0i:c0i + w]

            def csB(name):
                r, c0i, w = COLSB[name]
                return CB[0:r, c0i:c0i + w]

            identB = cst.tile([128, 128], BF16)
            nc.vector.tensor_scalar(out=identB[:], in0=csB("iota"),
                                    scalar1=csF("iotaC"), scalar2=None,
                                    op0=ALU.is_equal)
            ixl_t = cst.tile([128, 8 * TT], I16)
            nc.sync.dma_start(ixl_t[:], idx_xl[:])
            ixr_t = cst.tile([128, 8 * TT], I16)
            nc.sync.dma_start(ixr_t[:], idx_xr[:])
            dstl_t = cst.tile([128, TT], F32)
            nc.sync.dma_start(dstl_t[:], dstl[:])
            o_all = cst.tile([128, 2 * NBLK], F32)
            h0T = cst.tile([64, NPC], BF16)

            # prebuild all one-hot tiles into DRAM (reused by all 3 edge passes)
            for k0 in range(0, TT, 8):
                kc = min(8, TT - k0)
                bb = ohp.tile([128, 8, 128], BF16, tag="ohbuild")
                for j in range(kc):
                    nc.vector.tensor_scalar(
                        out=bb[:, j, :], in0=csB("iota"),
                        scalar1=dstl_t[:, k0 + j:k0 + j + 1], scalar2=None,
                        op0=ALU.is_equal)
                nc.sync.dma_start(ohs_d[:, k0 * 128:(k0 + kc) * 128],
                                  bb[:, :kc, :])

            # ---------------- phase A: L0 node prep ----------------
            for st, sz in chunks:
                xTc = sb.tile([D_IN, 128], BF16, tag="xTc")
                nc.sync.dma_start(xTc[:, :sz], xT[:, st:st + sz])
                p1 = psa.tile([64, 128], F32, tag="prep", space="PSUM")
                nc.tensor.matmul(p1[:, :sz], lhsT=csB("W_in"), rhs=xTc[:, :sz],
                                 start=True, stop=True)
                nc.scalar.activation(h0T[:, st:st + sz], p1[:, :sz], AF.Relu,
                                     bias=csF("b_in"))
                for W, tab in (("Wl0", xl0_own), ("Wr0", xr0)):
                    p2 = psa.tile([128, HC], F32, tag="prep", space="PSUM")
                    nc.tensor.matmul(p2[:sz, :], lhsT=h0T[:, st:st + sz],
                                     rhs=csB(W), start=True, stop=True)
                    cp = sb.tile([128, HC], BF16, tag="cpA")
                    nc.scalar.copy(cp[:sz, :], p2[:sz, :])
                    nc.sync.dma_start(tab[st:st + sz, :], cp[:sz, :])

            nc.gpsimd.collective_compute(
                "AllGather", ALU.bypass, ins=[xl0_own[:]], outs=[xl0_full[:]],
                replica_groups=rg)

            # ---------------- edge pass for layers 0/1 ----------------
            def edge_pass(lidx, xl_full, xr_tab, attB, aB, bB, hT_out,
                          h_own_out, residual):
                for b in range(NBLK):
                    st = b * BLOCK
                    nreal = min(BLOCK, NPC - st)
                    tl = blk_tiles[b]
                    T_all = sum(T for _, _, T in tl)
                    tglob0 = tl[0][0]
                    # gathers into one block-wide tile
                    gxl = gat.tile([128, TMAX, HC], BF16, tag="gxl")
                    tloc = 0
                    for (tg, nm, T) in tl:
                        src_ap = xl_full[0:HALF, :] if nm == "lo" else xl_full[HALF:N, :]
                        _gather(nc, gxl, src_ap, ixl_t, tg, tloc, T, HC)
                        tloc += T
                    gxr = gat.tile([128, TMAX, HC], BF16, tag="gxr")
                    _gather(nc, gxr, xr_tab[:], ixr_t, tglob0, 0, T_all, HC)
                    if residual is not None:
                        hres = sb.tile([128, HC], BF16, tag="hres")
                        if nreal < 128:
                            nc.vector.memset(hres[:], 0.0)
                        nc.sync.dma_start(hres[:nreal, :], residual[st:st + nreal, :])
                    # block-wide edge math (TT 2x / TS 4x ops)
                    u = eb1.tile([128, TMAX, HC], BF16, tag="u")
                    nc.vector.tensor_tensor(
                        out=u[:, :T_all, :], in0=gxl[:, :T_all, :],
                        in1=gxr[:, :T_all, :], op=ALU.add)
                    m = eb1.tile([128, TMAX, HC], BF16, tag="m")
                    nc.scalar.activation(m[:, :T_all, :], u[:, :T_all, :],
                                         AF.Prelu, alpha=NEG_SLOPE)
                    p = eb1.tile([128, TMAX, HID, HEADS], BF16, tag="p")
                    nc.vector.tensor_tensor(
                        out=p[:, :T_all, :, :].rearrange("a t c h -> a t (c h)"),
                        in0=m[:, :T_all, :],
                        in1=attB[:, None, :].to_broadcast([128, T_all, HC]),
                        op=ALU.mult)
                    # binary-tree per-head reduction (TT 2x, heads innermost)
                    q1 = eb1.tile([128, TMAX, 32, HEADS], BF16, tag="q1")
                    nc.vector.tensor_tensor(
                        out=q1[:, :T_all], in0=p[:, :T_all, 0:32, :],
                        in1=p[:, :T_all, 32:64, :], op=ALU.add)
                    q2 = eb1.tile([128, TMAX, 16, HEADS], BF16, tag="q2")
                    nc.vector.tensor_tensor(
                        out=q2[:, :T_all], in0=q1[:, :T_all, 0:16, :],
                        in1=q1[:, :T_all, 16:32, :], op=ALU.add)
                    q3 = eb1.tile([128, TMAX, 8, HEADS], BF16, tag="q3")
                    nc.vector.tensor_tensor(
                        out=q3[:, :T_all], in0=q2[:, :T_all, 0:8, :],
                        in1=q2[:, :T_all, 8:16, :], op=ALU.add)
                    q4 = eb1.tile([128, TMAX, 4, HEADS], BF16, tag="q4")
                    nc.vector.tensor_tensor(
                        out=q4[:, :T_all], in0=q3[:, :T_all, 0:4, :],
                        in1=q3[:, :T_all, 4:8, :], op=ALU.add)
                    q5 = eb1.tile([128, TMAX, 2, HEADS], BF16, tag="q5")
                    nc.vector.tensor_tensor(
                        out=q5[:, :T_all], in0=q4[:, :T_all, 0:2, :],
                        in1=q4[:, :T_all, 2:4, :], op=ALU.add)
                    lg = eb1.tile([128, TMAX, HEADS], F32, tag="lg")
                    nc.vector.tensor_tensor(
                        out=lg[:, :T_all], in0=q5[:, :T_all, 0, :],
                        in1=q5[:, :T_all, 1, :], op=ALU.add)
                    # combined scatter rhs: [ex-weighted xl | ex] per edge
                    rhsC = eb.tile([128, TMAX, HC + HEADS], BF16, tag="rhsC")
                    ex = rhsC[:, :, HC:HC + HEADS]
                    nc.scalar.activation(ex[:, :T_all], lg[:, :T_all], AF.Exp)
                    # block-wide ex premultiply (heads innermost -> packed 2x)
                    nc.vector.tensor_tensor(
                        out=rhsC[:, :T_all, 0:HC].rearrange("a t (c h) -> a t c h", h=HEADS),
                        in0=gxl[:, :T_all, :].rearrange("a t (c h) -> a t c h", h=HEADS),
                        in1=ex[:, :T_all, None, :].to_broadcast(
                            [128, T_all, HID, HEADS]),
                        op=ALU.mult)
                    # scatter: one matmul per tile (numerator + denominator)
                    acc = ps.tile([128, HC + HEADS], F32, tag="acc", space="PSUM")
                    oh_b = ohp.tile([128, TMAX, 128], BF16, tag="ohb")
                    nc.sync.dma_start(oh_b[:, :T_all, :].rearrange("a t d -> a (t d)"),
                                      ohs_d[:, tglob0 * 128:(tglob0 + T_all) * 128])
                    for t in range(T_all):
                        nc.tensor.matmul(
                            acc[:], lhsT=oh_b[:, t, :], rhs=rhsC[:, t, :],
                            start=(t == 0), stop=(t == T_all - 1))
                    # block post
                    rc = sb.tile([128, HEADS], F32, tag="rc")
                    nc.vector.reciprocal(rc[:], acc[:, HC:HC + HEADS])
                    go = sb.tile([128, HID, HEADS], BF16, tag="go")
                    nc.vector.tensor_tensor(
                        out=go[:], in0=acc[:, 0:HC].rearrange(
                            "a (c h) -> a c h", h=HEADS),
                        in1=rc[:, None, :].to_broadcast([128, HID, HEADS]),
                        op=ALU.mult)
                    t1 = sb.tile([128, HC], BF16, tag="t1")
                    nc.vector.tensor_tensor(
                        out=t1[:], in0=go[:].rearrange("a c h -> a (c h)"),
                        in1=aB, op=ALU.mult)
                    t2 = sb.tile([128, HC], BF16, tag="t2")
                    nc.vector.tensor_tensor(
                        out=t2[:], in0=t1[:], in1=bB, op=ALU.add)
                    h_t = sb.tile([128, HC], BF16, tag="h")
                    if residual is not None:
                        r_t = sb.tile([128, HC], BF16, tag="r")
                        nc.vector.tensor_scalar(
                            out=r_t[:], in0=t2[:], scalar1=0.0, scalar2=None,
                            op0=ALU.max)
                        nc.vector.tensor_tensor(
                            out=h_t[:], in0=r_t[:], in1=hres[:], op=ALU.add)
                    else:
                        nc.vector.tensor_scalar(
                            out=h_t[:], in0=t2[:], scalar1=0.0, scalar2=None,
                            op0=ALU.max)
                    if h_own_out is not None:
                        nc.sync.dma_start(h_own_out[st:st + nreal, :], h_t[:nreal, :])
                    for half in range(2):
                        tp = ps.tile([128, 128], BF16, tag="tp", space="PSUM")
                        nc.tensor.transpose(tp[:], h_t[:, half * 128:(half + 1) * 128],
                                            identB[:])
                        tcp = sb.tile([128, 128], BF16, tag="tcp")
                        nc.vector.tensor_copy(tcp[:], tp[:])
                        nc.sync.dma_start(hT_out[half * 128:(half + 1) * 128,
                                                 st:st + nreal], tcp[:, :nreal])

            edge_pass(0, xl0_full, xr0, csB("attB0"), csB("aB0"), csB("bB0"),
                      h1T, h1_own, None)

            # ---------------- phase C: L1 node prep ----------------
            for st, sz in chunks:
                ht0 = sb.tile([128, 128], BF16, tag="ht0")
                nc.sync.dma_start(ht0[:, :sz], h1T[0:128, st:st + sz])
                ht1 = sb.tile([128, 128], BF16, tag="ht1")
                nc.sync.dma_start(ht1[:, :sz], h1T[128:256, st:st + sz])
                for Wk0, Wk1, tab in (("Wl1k0", "Wl1k1", xl1_own),
                                      ("Wr1k0", "Wr1k1", xr1)):
                    p2 = psa.tile([128, HC], F32, tag="prep", space="PSUM")
                    nc.tensor.matmul(p2[:sz, :], lhsT=ht0[:, :sz], rhs=csB(Wk0),
                                     start=True, stop=False)
                    nc.tensor.matmul(p2[:sz, :], lhsT=ht1[:, :sz], rhs=csB(Wk1),
                                     start=False, stop=True)
                    cp = sb.tile([128, HC], BF16, tag="cpA")
                    nc.scalar.copy(cp[:sz, :], p2[:sz, :])
                    nc.sync.dma_start(tab[st:st + sz, :], cp[:sz, :])

            nc.gpsimd.collective_compute(
                "AllGather", ALU.bypass, ins=[xl1_own[:]], outs=[xl1_full[:]],
                replica_groups=rg)

            edge_pass(1, xl1_full, xr1, csB("attB1"), csB("aB1"), csB("bB1"),
                      h2T, None, h1_own)

            # ---------------- phase E: L2 node prep ----------------
            for st, sz in chunks:
                ht0 = sb.tile([128, 128], BF16, tag="ht0")
                nc.sync.dma_start(ht0[:, :sz], h2T[0:128, st:st + sz])
                ht1 = sb.tile([128, 128], BF16, tag="ht1")
                nc.sync.dma_start(ht1[:, :sz], h2T[128:256, st:st + sz])
                p2 = psa.tile([128, OUT], F32, tag="prep", space="PSUM")
                nc.tensor.matmul(p2[:sz, :], lhsT=ht0[:, :sz], rhs=csB("Wl2k0"),
                                 start=True, stop=False)
                nc.tensor.matmul(p2[:sz, :], lhsT=ht1[:, :sz], rhs=csB("Wl2k1"),
                                 start=False, stop=True)
                cp8 = sb.tile([128, 8], BF16, tag="cp8")
                nc.vector.memset(cp8[:], 0.0)
                nc.scalar.copy(cp8[:sz, 0:OUT], p2[:sz, :])
                nc.sync.dma_start(xl2p8_own[st:st + sz, :], cp8[:sz, :])
                p3 = psa.tile([128, OUT], F32, tag="prep", space="PSUM")
                nc.tensor.matmul(p3[:sz, :], lhsT=ht0[:, :sz], rhs=csB("Wr2k0"),
                                 start=True, stop=False)
                nc.tensor.matmul(p3[:sz, :], lhsT=ht1[:, :sz], rhs=csB("Wr2k1"),
                                 start=False, stop=True)
                cpr = sb.tile([128, 128], BF16, tag="cpr")
                nc.vector.memset(cpr[:], 0.0)
                nc.scalar.copy(cpr[:sz, 0:OUT], p3[:sz, :])
                nc.sync.dma_start(xr2p[st:st + sz, :], cpr[:sz, :])

            nc.gpsimd.collective_compute(
                "AllGather", ALU.bypass, ins=[xl2p8_own[:]], outs=[xl2p8_full[:]],
                replica_groups=rg)
            # expand [N, 8] -> cols 0:8 of [N, 128] (cols 8: stay garbage, unused)
            nc.sync.dma_start(xl2pB[:, 0:8], xl2p8_full[:])

            # ---------------- phase F: L2 edge pass ----------------
            att2 = csB("attB2")
            for b in range(NBLK):
                st = b * BLOCK
                nreal = min(BLOCK, NPC - st)
                tl = blk_tiles[b]
                T_all = sum(T for _, _, T in tl)
                tglob0 = tl[0][0]
                gxl = gat.tile([128, TMAX, 128], BF16, tag="gxl")
                tloc = 0
                for (tg, nm, T) in tl:
                    src_ap = xl2pB[0:HALF, :] if nm == "lo" else xl2pB[HALF:N, :]
                    _gather(nc, gxl, src_ap, ixl_t, tg, tloc, T, 128)
                    tloc += T
                gxr = gat.tile([128, TMAX, 128], BF16, tag="gxr")
                _gather(nc, gxr, xr2p[:], ixr_t, tglob0, 0, T_all, 128)
                u2 = eb1.tile([128, TMAX, OUT], BF16, tag="u2")
                nc.vector.scalar_tensor_tensor(
                    out=u2[:, :T_all, :], in0=gxl[:, :T_all, 0:OUT], scalar=1.0,
                    in1=gxr[:, :T_all, 0:OUT], op0=ALU.mult, op1=ALU.add)
                m2 = eb1.tile([128, TMAX, OUT], BF16, tag="m2")
                nc.vector.scalar_tensor_tensor(
                    out=m2[:, :T_all, :], in0=u2[:, :T_all, :], scalar=NEG_SLOPE,
                    in1=u2[:, :T_all, :], op0=ALU.mult, op1=ALU.max)
                p2_ = eb1.tile([128, TMAX, OUT], BF16, tag="p2")
                nc.vector.scalar_tensor_tensor(
                    out=p2_[:, :T_all, :], in0=m2[:, :T_all, :], scalar=1.0,
                    in1=att2[:, None, :].to_broadcast([128, T_all, OUT]),
                    op0=ALU.mult, op1=ALU.mult)
                lg2 = eb1.tile([128, TMAX], F32, tag="lg2")
                nc.vector.scalar_tensor_tensor(
                    out=lg2[:, :T_all], in0=p2_[:, :T_all, 0], scalar=1.0,
                    in1=p2_[:, :T_all, 1], op0=ALU.mult, op1=ALU.add)
                rhsC2 = eb.tile([128, TMAX, OUT + 1], BF16, tag="rhsC2")
                ex2 = rhsC2[:, :, OUT:OUT + 1]
                nc.scalar.activation(ex2[:, :T_all, 0], lg2[:, :T_all], AF.Exp)
                nc.vector.tensor_tensor(
                    out=rhsC2[:, :T_all, 0:OUT], in0=gxl[:, :T_all, 0:OUT],
                    in1=ex2[:, :T_all, :].to_broadcast([128, T_all, OUT]),
                    op=ALU.mult)
                acc2 = ps.tile([128, OUT + 1], F32, tag="acc", space="PSUM")
                oh_b = ohp.tile([128, TMAX, 128], BF16, tag="ohb")
                nc.sync.dma_start(oh_b[:, :T_all, :].rearrange("a t d -> a (t d)"),
                                  ohs_d[:, tglob0 * 128:(tglob0 + T_all) * 128])
                for t in range(T_all):
                    nc.tensor.matmul(acc2[:], lhsT=oh_b[:, t, :], rhs=rhsC2[:, t, :],
                                     start=(t == 0), stop=(t == T_all - 1))
                rc2 = sb.tile([128, 1], F32, tag="rc2")
                nc.vector.reciprocal(rc2[:], acc2[:, OUT:OUT + 1])
                o2 = sb.tile([128, OUT], F32, tag="o2")
                nc.vector.tensor_scalar(out=o2[:], in0=acc2[:, 0:OUT],
                                        scalar1=rc2[:], scalar2=None,
                                        op0=ALU.mult)
                nc.vector.tensor_tensor(out=o_all[:, 2 * b:2 * b + 2], in0=o2[:],
                                        in1=csF("bias2B"), op=ALU.add)

            # ---------------- phase G: log_softmax ----------------
            for b in range(NBLK):
                st = b * BLOCK
                nreal = min(BLOCK, NPC - st)
                d = sb.tile([128, 1], F32, tag="d")
                nc.vector.tensor_tensor(out=d[:], in0=o_all[:, 2 * b + 1:2 * b + 2],
                                        in1=o_all[:, 2 * b:2 * b + 1],
                                        op=ALU.subtract)
                e = sb.tile([128, 1], F32, tag="e")
                nc.scalar.activation(e[:], d[:], AF.Exp)
                ep1 = sb.tile([128, 1], F32, tag="ep1")
                nc.vector.tensor_scalar(out=ep1[:], in0=e[:], scalar1=1.0,
                                        scalar2=None, op0=ALU.add)
                l = sb.tile([128, 1], F32, tag="l")
                nc.scalar.activation(l[:], ep1[:], AF.Ln)
                ls = sb.tile([128, 2], F32, tag="ls")
                nc.vector.tensor_scalar(out=ls[:, 0:1], in0=l[:], scalar1=-1.0,
                                        scalar2=None, op0=ALU.mult)
                nc.vector.tensor_tensor(out=ls[:, 1:2], in0=d[:], in1=l[:],
                                        op=ALU.subtract)
                nc.sync.dma_start(out[st:st + nreal, :], ls[:nreal, :])

    nc.compile()
    return nc


COLSF = None
COLSB = None


# ---------------------------------------------------------------- entry
_CACHE = {}
LAST_RESULTS = None
LAST_NC = None


def kernel(**inputs):
    global COLSF, COLSB, LAST_RESULTS, LAST_NC
    x = np.asarray(inputs["x"], np.float32)
    ei = np.asarray(inputs["edge_index"]).astype(np.int64)
    N = x.shape[0]
    NPC = N // NCORES

    idx_xl, idx_xr, dstl, schedule, NBLK = preprocess(ei, N)
    TT = sum(T for _, _, T in schedule)
    cF, COLSF, cB, COLSB = pack_consts(inputs)

    key = (N, TT, NBLK, tuple(schedule))
    if key not in _CACHE:
        _CACHE[key] = build(N, schedule, NBLK, TT, cF.shape[1], cB.shape[1])
    nc = _CACHE[key]
    LAST_NC = nc

    in_maps = []
    for c in range(NCORES):
        sl = slice(c * NPC, (c + 1) * NPC)
        in_maps.append(dict(
            xT=np.ascontiguousarray(x[sl].T).astype(BFNP),
            idx_xl=idx_xl[c], idx_xr=idx_xr[c], dstl=dstl[c],
            constsF=cF, constsB=cB,
        ))
    res = run_bass_kernel_spmd(nc, in_maps, list(range(NCORES)))
    LAST_RESULTS = res
    outs = [res.results[c]["out"] for c in range(NCORES)]
    return np.concatenate(outs, axis=0).astype(np.float32)


# revision 7
# speedup vs baseline: 1.0527x; 1.0054x over previous
"""Trainium2 Bass kernel v2 for EnhancedGATModel (3-layer GATv2, N=50000, E=800000).

Strategy (8 NeuronCores, graph-partitioned by destination node):
- Host: append self-loops, sort edges by dst, partition dst nodes 6250/core,
  bucket edges per 128-dst block, split each block's edges by src half
  (int16 gather indices), pad to 128-edge tiles uniformly across cores.
- Device (single SPMD NEFF, all-bf16 tables):
  * per-layer node tables xl = h@Wl (own shard, bf16) + AllGather -> full table
  * per block: one dma_gather of xl[src] + xr[dst] rows into block-wide tiles,
    per-edge math (add/leaky/att-dot) as block-wide stt ops in bf16 (DVE 4x),
    per-head dot via binary-tree adds, exp batched per block,
    scatter via per-head ex-weighted one-hot matmuls accumulating in PSUM
    (denominator via ones-column matmul)
  * BN/relu/residual fused stt per node block; final log_softmax via Softplus.
"""
import sys
import numpy as np

sys.path.insert(0, "/opt/trn_rl_repo")

import ml_dtypes
import concourse.bass as bass
import concourse.mybir as mybir
import concourse.tile as tile
from concourse import bacc
from concourse.bass_utils import run_bass_kernel_spmd

F32 = mybir.dt.float32
BF16 = mybir.dt.bfloat16
I16 = mybir.dt.int16
AF = mybir.ActivationFunctionType
ALU = mybir.AluOpType
BFNP = ml_dtypes.bfloat16

NCORES = 8
BLOCK = 128
D_IN, HID, HEADS, OUT = 128, 64, 4, 2
HC = HEADS * HID  # 256
NEG_SLOPE = 0.2
BN_EPS = 1e-5
GMAX = 8  # dma_gather indices per op = 128*GMAX (HW limit 1024 idx)


# ---------------------------------------------------------------- host prep
def preprocess(edge_index, N):
    """Per-core gather index / dst-local arrays and the tile schedule.

    schedule: list of (block, 'lo'|'hi', ntiles) in tile order; uniform
    across cores. Edge k of a (core,block,half) group lands at partition
    k%128 of tile k//128; pads use src-index 0 (finite reads) and
    dst_local=200 (masked out of the one-hot).
    """
    NPC = N // NCORES
    NBLK = (NPC + BLOCK - 1) // BLOCK
    HALF = N // 2
    src = np.concatenate([edge_index[0], np.arange(N)]).astype(np.int64)
    dst = np.concatenate([edge_index[1], np.arange(N)]).astype(np.int64)
    order = np.argsort(dst, kind="stable")
    src, dst = src[order], dst[order]
    core_of = dst // NPC
    groups = {}
    for c in range(NCORES):
        m = core_of == c
        sc, dc = src[m], dst[m]
        loc = dc - c * NPC
        blk = loc // BLOCK
        lo = sc < HALF
        for b in range(NBLK):
            mb = blk == b
            groups[(c, b, 0)] = (sc[mb & lo], loc[mb & lo] % BLOCK)
            groups[(c, b, 1)] = (sc[mb & ~lo] - HALF, loc[mb & ~lo] % BLOCK)
    schedule = []
    for b in range(NBLK):
        for h, nm in ((0, "lo"), (1, "hi")):
            mx = max(len(groups[(c, b, h)][0]) for c in range(NCORES))
            T = (mx + 127) // 128
            if T > 0:
                schedule.append((b, nm, T))
    TT = sum(T for _, _, T in schedule)
    idx_xl = np.zeros((NCORES, 128, 8 * TT), np.int16)
    idx_xr = np.zeros((NCORES, 128, 8 * TT), np.int16)
    dstl = np.full((NCORES, 128, TT), 200.0, np.float32)
    t0 = 0
    for b, nm, T in schedule:
        h = 0 if nm == "lo" else 1
        for c in range(NCORES):
            s, dl = groups[(c, b, h)]
            ne = len(s)
            pad = T * 128 - ne
            sp = np.concatenate([s, np.zeros(pad, np.int64)]).astype(np.int64)
            dlp = np.concatenate([dl, np.full(pad, 200)]).astype(np.int64)
            wrap = sp.reshape(8 * T, 16).T.astype(np.int16)
            idx_xl[c, :, 8 * t0:8 * (t0 + T)] = np.tile(wrap, (8, 1))
            xr = b * BLOCK + np.minimum(dlp, BLOCK - 1)
            xr = np.minimum(xr, N // NCORES - 1)
            wrap2 = xr.reshape(8 * T, 16).T.astype(np.int16)
            idx_xr[c, :, 8 * t0:8 * (t0 + T)] = np.tile(wrap2, (8, 1))
            dstl[c, :, t0:t0 + T] = dlp.reshape(T, 128).T.astype(np.float32)
        t0 += T
    return idx_xl, idx_xr, dstl, schedule, NBLK


def _mkpack(dtype):
    cols = {}
    parts = []
    c0 = [0]

    def add(name, arr):
        a = np.zeros((128, arr.shape[1]), dtype)
        a[:arr.shape[0]] = arr.astype(np.float32)
        cols[name] = (arr.shape[0], c0[0], arr.shape[1])
        parts.append(a)
        c0[0] += arr.shape[1]

    return cols, parts, add


def pack_consts(ip):
    """Two packed const tensors: f32 and bf16."""
    f = lambda k: np.asarray(ip[k], np.float32)
    bcast = lambda v: np.broadcast_to(
        np.asarray(v, np.float32)[None, :], (128, len(np.asarray(v)))).copy()

    colsF, partsF, addF = _mkpack(np.float32)
    iota = np.broadcast_to(np.arange(128, dtype=np.float32), (128, 128))
    addF("iotaC", np.arange(128, dtype=np.float32)[:, None])
    addF("bias2B", bcast(f("bias2").reshape(-1)))
    addF("b_in", f("b_in").reshape(-1, 1))

    # column permutation: new j -> old h*HID+c with h=j%HEADS, c=j//HEADS
    PERM = np.array([(j % HEADS) * HID + (j // HEADS) for j in range(HC)])
    colsB, partsB, addB = _mkpack(BFNP)
    addB("iota", np.ascontiguousarray(iota))
    addB("one", np.ones((128, 1), np.float32))
    addB("attB0", bcast(f("att0").reshape(-1)[PERM]))
    addB("attB1", bcast(f("att1").reshape(-1)[PERM]))
    addB("attB2", bcast(f("att2").reshape(-1)))
    g, bt = f("bn_gamma"), f("bn_beta")
    mu, var = f("bn_mean"), f("bn_var")
    for l in range(2):
        a = g[l] / np.sqrt(var[l] + BN_EPS)
        b = bt[l] - mu[l] * a + a * f(f"bias{l}")
        addB(f"aB{l}", bcast(a[PERM]))
        addB(f"bB{l}", bcast(b[PERM]))
    addB("W_in", f("W_in"))
    addB("Wl0", f("Wl0")[:, PERM])
    addB("Wr0", f("Wr0")[:, PERM])
    Wl1, Wr1 = f("Wl1")[PERM][:, PERM], f("Wr1")[PERM][:, PERM]
    addB("Wl1k0", Wl1[:128]); addB("Wl1k1", Wl1[128:])
    addB("Wr1k0", Wr1[:128]); addB("Wr1k1", Wr1[128:])
    Wl2, Wr2 = f("Wl2")[PERM], f("Wr2")[PERM]
    addB("Wl2k0", Wl2[:128]); addB("Wl2k1", Wl2[128:])
    addB("Wr2k0", Wr2[:128]); addB("Wr2k1", Wr2[128:])
    cF = np.concatenate(partsF, axis=1)
    cB = np.concatenate(partsB, axis=1)
    return cF, colsF, cB, colsB


def _gather(nc, out_tile, in_ap, idx_tile, tstart, tout, T, elem):
    """Chunked dma_gather: out_tile[:, tout+k, :] = table[idx[tile tstart+k]]."""
    k = 0
    while k < T:
        Tc = min(GMAX, T - k)
        nc.gpsimd.dma_gather(
            out_ap=out_tile[:, tout + k:tout + k + Tc, :], in_ap=in_ap,
            idxs_ap=idx_tile[:, 8 * (tstart + k):8 * (tstart + k + Tc)],
            num_idxs=128 * Tc, num_idxs_reg=128 * Tc, elem_size=elem)
        k += Tc


def _chunks(NPC):
    out = []
    st = 0
    while st < NPC:
        out.append((st, min(128, NPC - st)))
        st += 128
    return out


def build(N, schedule, NBLK, TT, CWF, CWB):
    NPC = N // NCORES
    HALF = N // 2
    nc = bacc.Bacc("TRN2", target_bir_lowering=False, debug=False)

    xT = nc.dram_tensor("xT", [D_IN, NPC], BF16, kind="ExternalInput")
    idx_xl = nc.dram_tensor("idx_xl", [128, 8 * TT], I16, kind="ExternalInput")
    idx_xr = nc.dram_tensor("idx_xr", [128, 8 * TT], I16, kind="ExternalInput")
    dstl = nc.dram_tensor("dstl", [128, TT], F32, kind="ExternalInput")
    constsF = nc.dram_tensor("constsF", [128, CWF], F32, kind="ExternalInput")
    constsB = nc.dram_tensor("constsB", [128, CWB], BF16, kind="ExternalInput")
    out = nc.dram_tensor("out", [NPC, OUT], F32, kind="ExternalOutput")

    ohs_d = nc.dram_tensor("ohs_d", [128, TT * 128], BF16)
    xl0_own = nc.dram_tensor("xl0_own", [NPC, HC], BF16)
    xl0_full = nc.dram_tensor("xl0_full", [N, HC], BF16, addr_space="Shared")
    xr0 = nc.dram_tensor("xr0", [NPC, HC], BF16)
    h1_own = nc.dram_tensor("h1_own", [NPC, HC], BF16)
    h1T = nc.dram_tensor("h1T", [HC, NPC], BF16)
    xl1_own = nc.dram_tensor("xl1_own", [NPC, HC], BF16)
    xl1_full = nc.dram_tensor("xl1_full", [N, HC], BF16, addr_space="Shared")
    xr1 = nc.dram_tensor("xr1", [NPC, HC], BF16)
    h2T = nc.dram_tensor("h2T", [HC, NPC], BF16)
    xl2p8_own = nc.dram_tensor("xl2p8_own", [NPC, 8], BF16)
    xl2p8_full = nc.dram_tensor("xl2p8_full", [N, 8], BF16, addr_space="Shared")
    xl2pB = nc.dram_tensor("xl2pB", [N, 128], BF16)  # cols 0:8 valid
    xr2p = nc.dram_tensor("xr2p", [NPC, 128], BF16)  # cols 0:2 valid

    chunks = _chunks(NPC)
    blk_tiles = {b: [] for b in range(NBLK)}
    t0 = 0
    for b, nm, T in schedule:
        blk_tiles[b].append((t0, nm, T))
        t0 += T
    TMAX = max(sum(T for _, _, T in blk_tiles[b]) for b in range(NBLK))

    rg = [list(range(NCORES))]

    with tile.TileContext(nc) as tc:
        import contextlib
        with contextlib.ExitStack() as ctx:
            cst = ctx.enter_context(tc.tile_pool(name="cst", bufs=1))
            sb = ctx.enter_context(tc.tile_pool(name="sb", bufs=3))
            eb = ctx.enter_context(tc.tile_pool(name="eb", bufs=2))
            eb1 = ctx.enter_context(tc.tile_pool(name="eb1", bufs=1))
            gat = ctx.enter_context(tc.tile_pool(name="gat", bufs=2))
            ohp = ctx.enter_context(tc.tile_pool(name="ohp", bufs=2))
            ps = ctx.enter_context(tc.tile_pool(name="ps", bufs=2, space="PSUM"))
            psa = ctx.enter_context(tc.tile_pool(name="psa", bufs=2, space="PSUM"))

            CF = cst.tile([128, CWF], F32)
            nc.sync.dma_start(CF[:], constsF[:])
            CB = cst.tile([128, CWB], BF16)
            nc.sync.dma_start(CB[:], constsB[:])

            def csF(name):
                r, c0i, w = COLSF[name]
                return CF[0:r, c0i:c0i + w]

            def csB(name):
                r, c0i, w = COLSB[name]
                return CB[0:r, c0i:c0i + w]

            identB = cst.tile([128, 128], BF16)
            nc.vector.tensor_scalar(out=identB[:], in0=csB("iota"),
                                    scalar1=csF("iotaC"), scalar2=None,
                                    op0=ALU.is_equal)
            ixl_t = cst.tile([128, 8 * TT], I16)
            nc.sync.dma_start(ixl_t[:], idx_xl[:])
            ixr_t = cst.tile([128, 8 * TT], I16)
            nc.sync.dma_start(ixr_t[:], idx_xr[:])
            dstl_t = cst.tile([128, TT], F32)
            nc.sync.dma_start(dstl_t[:], dstl[:])
            o_all = cst.tile([128, 2 * NBLK], F32)
            h0T = cst.tile([64, NPC], BF16)

            # prebuild all one-hot tiles into DRAM (reused by all 3 edge passes)
            for k0 in range(0, TT, 8):
                kc = min(8, TT - k0)
                bb = ohp.tile([128, 8, 128], BF16, tag="ohbuild")
                for j in range(kc):
                    nc.vector.tensor_scalar(
                        out=bb[:, j, :], in0=csB("iota"),
                        scalar1=dstl_t[:, k0 + j:k0 + j + 1], scalar2=None,
                        op0=ALU.is_equal)
                nc.sync.dma_start(ohs_d[:, k0 * 128:(k0 + kc) * 128],
                                  bb[:, :kc, :])

            # ---------------- phase A: L0 node prep ----------------
            for st, sz in chunks:
                xTc = sb.tile([D_IN, 128], BF16, tag="xTc")
                nc.sync.dma_start(xTc[:, :sz], xT[:, st:st + sz])
                p1 = psa.tile([64, 128], F32, tag="prep", space="PSUM")
                nc.tensor.matmul(p1[:, :sz], lhsT=csB("W_in"), rhs=xTc[:, :sz],
                                 start=True, stop=True)
                nc.scalar.activation(h0T[:, st:st + sz], p1[:, :sz], AF.Relu,
                                     bias=csF("b_in"))
                for W, tab in (("Wl0", xl0_own), ("Wr0", xr0)):
                    p2 = psa.tile([128, HC], F32, tag="prep", space="PSUM")
                    nc.tensor.matmul(p2[:sz, :], lhsT=h0T[:, st:st + sz],
                                     rhs=csB(W), start=True, stop=True)
                    cp = sb.tile([128, HC], BF16, tag="cpA")
                    nc.scalar.copy(cp[:sz, :], p2[:sz, :])
                    nc.sync.dma_start(tab[st:st + sz, :], cp[:sz, :])

            nc.gpsimd.collective_compute(
                "AllGather", ALU.bypass, ins=[xl0_own[:]], outs=[xl0_full[:]],
                replica_groups=rg)

            # ---------------- edge pass for layers 0/1 ----------------
            def edge_pass(lidx, xl_full, xr_tab, attB, aB, bB, hT_out,
                          h_own_out, residual, prep_fn=None):
                for b in range(NBLK):
                    st = b * BLOCK
                    nreal = min(BLOCK, NPC - st)
                    tl = blk_tiles[b]
                    T_all = sum(T for _, _, T in tl)
                    tglob0 = tl[0][0]
                    # gathers into one block-wide tile
                    gxl = gat.tile([128, TMAX, HC], BF16, tag="gxl")
                    tloc = 0
                    for (tg, nm, T) in tl:
                        src_ap = xl_full[0:HALF, :] if nm == "lo" else xl_full[HALF:N, :]
                        _gather(nc, gxl, src_ap, ixl_t, tg, tloc, T, HC)
                        tloc += T
                    gxr = gat.tile([128, TMAX, HC], BF16, tag="gxr")
                    _gather(nc, gxr, xr_tab[:], ixr_t, tglob0, 0, T_all, HC)
                    if residual is not None:
                        hres = sb.tile([128, HC], BF16, tag="hres")
                        if nreal < 128:
                            nc.vector.memset(hres[:], 0.0)
                        nc.sync.dma_start(hres[:nreal, :], residual[st:st + nreal, :])
                    # block-wide edge math (TT 2x / TS 4x ops)
                    u = eb1.tile([128, TMAX, HC], BF16, tag="u")
                    nc.vector.tensor_tensor(
                        out=u[:, :T_all, :], in0=gxl[:, :T_all, :],
                        in1=gxr[:, :T_all, :], op=ALU.add)
                    m = eb1.tile([128, TMAX, HC], BF16, tag="m")
                    nc.scalar.activation(m[:, :T_all, :], u[:, :T_all, :],
                                         AF.Prelu, alpha=NEG_SLOPE)
                    p = eb1.tile([128, TMAX, HID, HEADS], BF16, tag="p")
                    nc.vector.tensor_tensor(
                        out=p[:, :T_all, :, :].rearrange("a t c h -> a t (c h)"),
                        in0=m[:, :T_all, :],
                        in1=attB[:, None, :].to_broadcast([128, T_all, HC]),
                        op=ALU.mult)
                    # binary-tree per-head reduction (TT 2x, heads innermost)
                    q1 = eb1.tile([128, TMAX, 32, HEADS], BF16, tag="q1")
                    nc.vector.tensor_tensor(
                        out=q1[:, :T_all], in0=p[:, :T_all, 0:32, :],
                        in1=p[:, :T_all, 32:64, :], op=ALU.add)
                    q2 = eb1.tile([128, TMAX, 16, HEADS], BF16, tag="q2")
                    nc.vector.tensor_tensor(
                        out=q2[:, :T_all], in0=q1[:, :T_all, 0:16, :],
                        in1=q1[:, :T_all, 16:32, :], op=ALU.add)
                    q3 = eb1.tile([128, TMAX, 8, HEADS], BF16, tag="q3")
                    nc.vector.tensor_tensor(
                        out=q3[:, :T_all], in0=q2[:, :T_all, 0:8, :],
                        in1=q2[:, :T_all, 8:16, :], op=ALU.add)
                    q4 = eb1.tile([128, TMAX, 4, HEADS], BF16, tag="q4")
                    nc.vector.tensor_tensor(
                        out=q4[:, :T_all], in0=q3[:, :T_all, 0:4, :],
                        in1=q3[:, :T_all, 4:8, :], op=ALU.add)
                    q5 = eb1.tile([128, TMAX, 2, HEADS], BF16, tag="q5")
                    nc.vector.tensor_tensor(
                        out=q5[:, :T_all], in0=q4[:, :T_all, 0:2, :],
                        in1=q4[:, :T_all, 2:4, :], op=ALU.add)
                    lg = eb1.tile([128, TMAX, HEADS], F32, tag="lg")
                    nc.vector.tensor_tensor(
                        out=lg[:, :T_all], in0=q5[:, :T_all, 0, :],
                        in1=q5[:, :T_all, 1, :], op=ALU.add)
                    # combined scatter rhs: [ex-weighted xl | ex] per edge
                    rhsC = eb.tile([128, TMAX, HC + HEADS], BF16, tag="rhsC")
                    ex = rhsC[:, :, HC:HC + HEADS]
                    nc.scalar.activation(ex[:, :T_all], lg[:, :T_all], AF.Exp)
                    # block-wide ex premultiply (heads innermost -> packed 2x)
                    nc.vector.tensor_tensor(
                        out=rhsC[:, :T_all, 0:HC].rearrange("a t (c h) -> a t c h", h=HEADS),
                        in0=gxl[:, :T_all, :].rearrange("a t (c h) -> a t c h", h=HEADS),
                        in1=ex[:, :T_all, None, :].to_broadcast(
                            [128, T_all, HID, HEADS]),
                        op=ALU.mult)
                    # scatter: one matmul per tile (numerator + denominator)
                    acc = ps.tile([128, HC + HEADS], F32, tag="acc", space="PSUM")
                    oh_b = ohp.tile([128, TMAX, 128], BF16, tag="ohb")
                    nc.sync.dma_start(oh_b[:, :T_all, :].rearrange("a t d -> a (t d)"),
                                      ohs_d[:, tglob0 * 128:(tglob0 + T_all) * 128])
                    for t in range(T_all):
                        nc.tensor.matmul(
                            acc[:], lhsT=oh_b[:, t, :], rhs=rhsC[:, t, :],
                            start=(t == 0), stop=(t == T_all - 1))
                    # block post
                    rc = sb.tile([128, HEADS], F32, tag="rc")
                    nc.vector.reciprocal(rc[:], acc[:, HC:HC + HEADS])
                    go = sb.tile([128, HID, HEADS], BF16, tag="go")
                    nc.vector.tensor_tensor(
                        out=go[:], in0=acc[:, 0:HC].rearrange(
                            "a (c h) -> a c h", h=HEADS),
                        in1=rc[:, None, :].to_broadcast([128, HID, HEADS]),
                        op=ALU.mult)
                    t1 = sb.tile([128, HC], BF16, tag="t1")
                    nc.vector.tensor_tensor(
                        out=t1[:], in0=go[:].rearrange("a c h -> a (c h)"),
                        in1=aB, op=ALU.mult)
                    t2 = sb.tile([128, HC], BF16, tag="t2")
                    nc.vector.tensor_tensor(
                        out=t2[:], in0=t1[:], in1=bB, op=ALU.add)
                    h_t = sb.tile([128, HC], BF16, tag="h")
                    if residual is not None:
                        r_t = sb.tile([128, HC], BF16, tag="r")
                        nc.vector.tensor_scalar(
                            out=r_t[:], in0=t2[:], scalar1=0.0, scalar2=None,
                            op0=ALU.max)
                        nc.vector.tensor_tensor(
                            out=h_t[:], in0=r_t[:], in1=hres[:], op=ALU.add)
                    else:
                        nc.vector.tensor_scalar(
                            out=h_t[:], in0=t2[:], scalar1=0.0, scalar2=None,
                            op0=ALU.max)
                    if h_own_out is not None:
                        nc.sync.dma_start(h_own_out[st:st + nreal, :], h_t[:nreal, :])
                    tcps = []
                    for half in range(2):
                        tp = ps.tile([128, 128], BF16, tag="tp", space="PSUM")
                        nc.tensor.transpose(tp[:], h_t[:, half * 128:(half + 1) * 128],
                                            identB[:])
                        tcp = sb.tile([128, 128], BF16, tag=f"tcp{half}")
                        nc.vector.tensor_copy(tcp[:], tp[:])
                        tcps.append(tcp)
                    # fused node prep for the next layer (no DRAM round trip)
                    prep_fn(st, nreal, tcps)

            def prep_l1(st, sz, tcps):
                for Wk0, Wk1, tab in (("Wl1k0", "Wl1k1", xl1_own),
                                      ("Wr1k0", "Wr1k1", xr1)):
                    p2 = psa.tile([128, HC], F32, tag="prep", space="PSUM")
                    nc.tensor.matmul(p2[:sz, :], lhsT=tcps[0][:, :sz], rhs=csB(Wk0),
                                     start=True, stop=False)
                    nc.tensor.matmul(p2[:sz, :], lhsT=tcps[1][:, :sz], rhs=csB(Wk1),
                                     start=False, stop=True)
                    cp = sb.tile([128, HC], BF16, tag="cpA")
                    nc.scalar.copy(cp[:sz, :], p2[:sz, :])
                    nc.sync.dma_start(tab[st:st + sz, :], cp[:sz, :])

            edge_pass(0, xl0_full, xr0, csB("attB0"), csB("aB0"), csB("bB0"),
                      h1T, h1_own, None, prep_l1)

            nc.gpsimd.collective_compute(
                "AllGather", ALU.bypass, ins=[xl1_own[:]], outs=[xl1_full[:]],
                replica_groups=rg)

            def prep_l2(st, sz, tcps):
                p2 = psa.tile([128, OUT], F32, tag="prep", space="PSUM")
                nc.tensor.matmul(p2[:sz, :], lhsT=tcps[0][:, :sz], rhs=csB("Wl2k0"),
                                 start=True, stop=False)
                nc.tensor.matmul(p2[:sz, :], lhsT=tcps[1][:, :sz], rhs=csB("Wl2k1"),
                                 start=False, stop=True)
                cp8 = sb.tile([128, 8], BF16, tag="cp8")
                nc.vector.memset(cp8[:], 0.0)
                nc.scalar.copy(cp8[:sz, 0:OUT], p2[:sz, :])
                nc.sync.dma_start(xl2p8_own[st:st + sz, :], cp8[:sz, :])
                p3 = psa.tile([128, OUT], F32, tag="prep", space="PSUM")
                nc.tensor.matmul(p3[:sz, :], lhsT=tcps[0][:, :sz], rhs=csB("Wr2k0"),
                                 start=True, stop=False)
                nc.tensor.matmul(p3[:sz, :], lhsT=tcps[1][:, :sz], rhs=csB("Wr2k1"),
                                 start=False, stop=True)
                cpr = sb.tile([128, 128], BF16, tag="cpr")
                nc.vector.memset(cpr[:], 0.0)
                nc.scalar.copy(cpr[:sz, 0:OUT], p3[:sz, :])
                nc.sync.dma_start(xr2p[st:st + sz, :], cpr[:sz, :])

            edge_pass(1, xl1_full, xr1, csB("attB1"), csB("aB1"), csB("bB1"),
                      h2T, None, h1_own, prep_l2)

            nc.gpsimd.collective_compute(
                "AllGather", ALU.bypass, ins=[xl2p8_own[:]], outs=[xl2p8_full[:]],
                replica_groups=rg)
            # expand [N, 8] -> cols 0:8 of [N, 128] (cols 8: stay garbage, unused)
            nc.sync.dma_start(xl2pB[:, 0:8], xl2p8_full[:])

            # ---------------- phase F: L2 edge pass ----------------
            att2 = csB("attB2")
            for b in range(NBLK):
                st = b * BLOCK
                nreal = min(BLOCK, NPC - st)
                tl = blk_tiles[b]
                T_all = sum(T for _, _, T in tl)
                tglob0 = tl[0][0]
                gxl = gat.tile([128, TMAX, 128], BF16, tag="gxl")
                tloc = 0
                for (tg, nm, T) in tl:
                    src_ap = xl2pB[0:HALF, :] if nm == "lo" else xl2pB[HALF:N, :]
                    _gather(nc, gxl, src_ap, ixl_t, tg, tloc, T, 128)
                    tloc += T
                gxr = gat.tile([128, TMAX, 128], BF16, tag="gxr")
                _gather(nc, gxr, xr2p[:], ixr_t, tglob0, 0, T_all, 128)
                u2 = eb1.tile([128, TMAX, OUT], BF16, tag="u2")
                nc.vector.scalar_tensor_tensor(
                    out=u2[:, :T_all, :], in0=gxl[:, :T_all, 0:OUT], scalar=1.0,
                    in1=gxr[:, :T_all, 0:OUT], op0=ALU.mult, op1=ALU.add)
                m2 = eb1.tile([128, TMAX, OUT], BF16, tag="m2")
                nc.vector.scalar_tensor_tensor(
                    out=m2[:, :T_all, :], in0=u2[:, :T_all, :], scalar=NEG_SLOPE,
                    in1=u2[:, :T_all, :], op0=ALU.mult, op1=ALU.max)
                p2_ = eb1.tile([128, TMAX, OUT], BF16, tag="p2")
                nc.vector.scalar_tensor_tensor(
                    out=p2_[:, :T_all, :], in0=m2[:, :T_all, :], scalar=1.0,
                    in1=att2[:, None, :].to_broadcast([128, T_all, OUT]),
                    op0=ALU.mult, op1=ALU.mult)
                lg2 = eb1.tile([128, TMAX], F32, tag="lg2")
                nc.vector.scalar_tensor_tensor(
                    out=lg2[:, :T_all], in0=p2_[:, :T_all, 0], scalar=1.0,
                    in1=p2_[:, :T_all, 1], op0=ALU.mult, op1=ALU.add)
                rhsC2 = eb.tile([128, TMAX, OUT + 1], BF16, tag="rhsC2")
                ex2 = rhsC2[:, :, OUT:OUT + 1]
                nc.scalar.activation(ex2[:, :T_all, 0], lg2[:, :T_all], AF.Exp)
                nc.vector.tensor_tensor(
                    out=rhsC2[:, :T_all, 0:OUT], in0=gxl[:, :T_all, 0:OUT],
                    in1=ex2[:, :T_all, :].to_broadcast([128, T_all, OUT]),
                    op=ALU.mult)
                acc2 = ps.tile([128, OUT + 1], F32, tag="acc", space="PSUM")
                oh_b = ohp.tile([128, TMAX, 128], BF16, tag="ohb")
                nc.sync.dma_start(oh_b[:, :T_all, :].rearrange("a t d -> a (t d)"),
                                  ohs_d[:, tglob0 * 128:(tglob0 + T_all) * 128])
                for t in range(T_all):
                    nc.tensor.matmul(acc2[:], lhsT=oh_b[:, t, :], rhs=rhsC2[:, t, :],
                                     start=(t == 0), stop=(t == T_all - 1))
                rc2 = sb.tile([128, 1], F32, tag="rc2")
                nc.vector.reciprocal(rc2[:], acc2[:, OUT:OUT + 1])
                o2 = sb.tile([128, OUT], F32, tag="o2")
                nc.vector.tensor_scalar(out=o2[:], in0=acc2[:, 0:OUT],
                                        scalar1=rc2[:], scalar2=None,
                                        op0=ALU.mult)
                nc.vector.tensor_tensor(out=o_all[:, 2 * b:2 * b + 2], in0=o2[:],
                                        in1=csF("bias2B"), op=ALU.add)

            # ---------------- phase G: log_softmax ----------------
            for b in range(NBLK):
                st = b * BLOCK
                nreal = min(BLOCK, NPC - st)
                d = sb.tile([128, 1], F32, tag="d")
                nc.vector.tensor_tensor(out=d[:], in0=o_all[:, 2 * b + 1:2 * b + 2],
                                        in1=o_all[:, 2 * b:2 * b + 1],
                                        op=ALU.subtract)
                e = sb.tile([128, 1], F32, tag="e")
                nc.scalar.activation(e[:], d[:], AF.Exp)
                ep1 = sb.tile([128, 1], F32, tag="ep1")
                nc.vector.tensor_scalar(out=ep1[:], in0=e[:], scalar1=1.0,
                                        scalar2=None, op0=ALU.add)
                l = sb.tile([128, 1], F32, tag="l")
                nc.scalar.activation(l[:], ep1[:], AF.Ln)
                ls = sb.tile([128, 2], F32, tag="ls")
                nc.vector.tensor_scalar(out=ls[:, 0:1], in0=l[:], scalar1=-1.0,
                                        scalar2=None, op0=ALU.mult)
                nc.vector.tensor_tensor(out=ls[:, 1:2], in0=d[:], in1=l[:],
                                        op=ALU.subtract)
                nc.sync.dma_start(out[st:st + nreal, :], ls[:nreal, :])

    nc.compile()
    return nc


COLSF = None
COLSB = None


# ---------------------------------------------------------------- entry
_CACHE = {}
LAST_RESULTS = None
LAST_NC = None


def kernel(**inputs):
    global COLSF, COLSB, LAST_RESULTS, LAST_NC
    x = np.asarray(inputs["x"], np.float32)
    ei = np.asarray(inputs["edge_index"]).astype(np.int64)
    N = x.shape[0]
    NPC = N // NCORES

    idx_xl, idx_xr, dstl, schedule, NBLK = preprocess(ei, N)
    TT = sum(T for _, _, T in schedule)
    cF, COLSF, cB, COLSB = pack_consts(inputs)

    key = (N, TT, NBLK, tuple(schedule))
    if key not in _CACHE:
        _CACHE[key] = build(N, schedule, NBLK, TT, cF.shape[1], cB.shape[1])
    nc = _CACHE[key]
    LAST_NC = nc

    in_maps = []
    for c in range(NCORES):
        sl = slice(c * NPC, (c + 1) * NPC)
        in_maps.append(dict(
            xT=np.ascontiguousarray(x[sl].T).astype(BFNP),
            idx_xl=idx_xl[c], idx_xr=idx_xr[c], dstl=dstl[c],
            constsF=cF, constsB=cB,
        ))
    res = run_bass_kernel_spmd(nc, in_maps, list(range(NCORES)))
    LAST_RESULTS = res
    outs = [res.results[c]["out"] for c in range(NCORES)]
    return np.concatenate(outs, axis=0).astype(np.float32)


# revision 11
# speedup vs baseline: 1.1202x; 1.0641x over previous
"""Trainium2 Bass kernel v2 for EnhancedGATModel (3-layer GATv2, N=50000, E=800000).

Strategy (8 NeuronCores, graph-partitioned by destination node):
- Host: append self-loops, sort edges by dst, partition dst nodes 6250/core,
  bucket edges per 128-dst block, split each block's edges by src half
  (int16 gather indices), pad to 128-edge tiles uniformly across cores.
- Device (single SPMD NEFF, all-bf16 tables):
  * per-layer node tables xl = h@Wl (own shard, bf16) + AllGather -> full table
  * per block: one dma_gather of xl[src] + xr[dst] rows into block-wide tiles,
    per-edge math (add/leaky/att-dot) as block-wide stt ops in bf16 (DVE 4x),
    per-head dot via binary-tree adds, exp batched per block,
    scatter via per-head ex-weighted one-hot matmuls accumulating in PSUM
    (denominator via ones-column matmul)
  * BN/relu/residual fused stt per node block; final log_softmax via Softplus.
"""
import sys
import numpy as np

sys.path.insert(0, "/opt/trn_rl_repo")

import ml_dtypes
import concourse.bass as bass
import concourse.mybir as mybir
import concourse.tile as tile
from concourse import bacc
from concourse.bass_utils import run_bass_kernel_spmd

F32 = mybir.dt.float32
BF16 = mybir.dt.bfloat16
I16 = mybir.dt.int16
AF = mybir.ActivationFunctionType
ALU = mybir.AluOpType
BFNP = ml_dtypes.bfloat16

NCORES = 8
BLOCK = 128
D_IN, HID, HEADS, OUT = 128, 64, 4, 2
HC = HEADS * HID  # 256
NEG_SLOPE = 0.2
BN_EPS = 1e-5
GMAX = 8  # dma_gather indices per op = 128*GMAX (HW limit 1024 idx)


# ---------------------------------------------------------------- host prep
def preprocess(edge_index, N):
    """Per-core gather index / dst-local arrays and the tile schedule.

    schedule: list of (block, 'lo'|'hi', ntiles) in tile order; uniform
    across cores. Edge k of a (core,block,half) group lands at partition
    k%128 of tile k//128; pads use src-index 0 (finite reads) and
    dst_local=200 (masked out of the one-hot).
    """
    NPC = N // NCORES
    NBLK = (NPC + BLOCK - 1) // BLOCK
    HALF = N // 2
    src = np.concatenate([edge_index[0], np.arange(N)]).astype(np.int64)
    dst = np.concatenate([edge_index[1], np.arange(N)]).astype(np.int64)
    order = np.argsort(dst, kind="stable")
    src, dst = src[order], dst[order]
    core_of = dst // NPC
    groups = {}
    for c in range(NCORES):
        m = core_of == c
        sc, dc = src[m], dst[m]
        loc = dc - c * NPC
        blk = loc // BLOCK
        lo = sc < HALF
        for b in range(NBLK):
            mb = blk == b
            groups[(c, b, 0)] = (sc[mb & lo], loc[mb & lo] % BLOCK)
            groups[(c, b, 1)] = (sc[mb & ~lo] - HALF, loc[mb & ~lo] % BLOCK)
    schedule = []
    for b in range(NBLK):
        for h, nm in ((0, "lo"), (1, "hi")):
            mx = max(len(groups[(c, b, h)][0]) for c in range(NCORES))
            T = (mx + 127) // 128
            if T > 0:
                schedule.append((b, nm, T))
    TT = sum(T for _, _, T in schedule)
    idx_xl = np.zeros((NCORES, 128, 8 * TT), np.int16)
    idx_xr = np.zeros((NCORES, 128, 8 * TT), np.int16)
    dstl = np.full((NCORES, 128, TT), 200.0, np.float32)
    t0 = 0
    for b, nm, T in schedule:
        h = 0 if nm == "lo" else 1
        for c in range(NCORES):
            s, dl = groups[(c, b, h)]
            ne = len(s)
            pad = T * 128 - ne
            sp = np.concatenate([s, np.zeros(pad, np.int64)]).astype(np.int64)
            dlp = np.concatenate([dl, np.full(pad, 200)]).astype(np.int64)
            wrap = sp.reshape(8 * T, 16).T.astype(np.int16)
            idx_xl[c, :, 8 * t0:8 * (t0 + T)] = np.tile(wrap, (8, 1))
            xr = b * BLOCK + np.minimum(dlp, BLOCK - 1)
            xr = np.minimum(xr, N // NCORES - 1)
            wrap2 = xr.reshape(8 * T, 16).T.astype(np.int16)
            idx_xr[c, :, 8 * t0:8 * (t0 + T)] = np.tile(wrap2, (8, 1))
            dstl[c, :, t0:t0 + T] = dlp.reshape(T, 128).T.astype(np.float32)
        t0 += T
    return idx_xl, idx_xr, dstl, schedule, NBLK


def _mkpack(dtype):
    cols = {}
    parts = []
    c0 = [0]

    def add(name, arr):
        a = np.zeros((128, arr.shape[1]), dtype)
        a[:arr.shape[0]] = arr.astype(np.float32)
        cols[name] = (arr.shape[0], c0[0], arr.shape[1])
        parts.append(a)
        c0[0] += arr.shape[1]

    return cols, parts, add


def pack_consts(ip):
    """Two packed const tensors: f32 and bf16."""
    f = lambda k: np.asarray(ip[k], np.float32)
    bcast = lambda v: np.broadcast_to(
        np.asarray(v, np.float32)[None, :], (128, len(np.asarray(v)))).copy()

    colsF, partsF, addF = _mkpack(np.float32)
    iota = np.broadcast_to(np.arange(128, dtype=np.float32), (128, 128))
    addF("iotaC", np.arange(128, dtype=np.float32)[:, None])
    addF("bias2B", bcast(f("bias2").reshape(-1)))
    addF("b_in", f("b_in").reshape(-1, 1))

    # column permutation: new j -> old h*HID+c with h=j%HEADS, c=j//HEADS
    PERM = np.array([(j % HEADS) * HID + (j // HEADS) for j in range(HC)])
    colsB, partsB, addB = _mkpack(BFNP)
    addB("iota", np.ascontiguousarray(iota))
    addB("one", np.ones((128, 1), np.float32))
    addB("attB0", bcast(f("att0").reshape(-1)[PERM]))
    addB("attB1", bcast(f("att1").reshape(-1)[PERM]))
    addB("attB2", bcast(f("att2").reshape(-1)))
    g, bt = f("bn_gamma"), f("bn_beta")
    mu, var = f("bn_mean"), f("bn_var")
    for l in range(2):
        a = g[l] / np.sqrt(var[l] + BN_EPS)
        b = bt[l] - mu[l] * a + a * f(f"bias{l}")
        addB(f"aB{l}", bcast(a[PERM]))
        addB(f"bB{l}", bcast(b[PERM]))
    addB("W_in", f("W_in"))
    addB("Wl0", f("Wl0")[:, PERM])
    addB("Wr0", f("Wr0")[:, PERM])
    Wl1, Wr1 = f("Wl1")[PERM][:, PERM], f("Wr1")[PERM][:, PERM]
    addB("Wl1k0", Wl1[:128]); addB("Wl1k1", Wl1[128:])
    addB("Wr1k0", Wr1[:128]); addB("Wr1k1", Wr1[128:])
    Wl2, Wr2 = f("Wl2")[PERM], f("Wr2")[PERM]
    addB("Wl2k0", Wl2[:128]); addB("Wl2k1", Wl2[128:])
    addB("Wr2k0", Wr2[:128]); addB("Wr2k1", Wr2[128:])
    cF = np.concatenate(partsF, axis=1)
    cB = np.concatenate(partsB, axis=1)
    return cF, colsF, cB, colsB


def _gather(nc, out_tile, in_ap, idx_tile, tstart, tout, T, elem):
    """Chunked dma_gather: out_tile[:, tout+k, :] = table[idx[tile tstart+k]]."""
    k = 0
    while k < T:
        Tc = min(GMAX, T - k)
        nc.gpsimd.dma_gather(
            out_ap=out_tile[:, tout + k:tout + k + Tc, :], in_ap=in_ap,
            idxs_ap=idx_tile[:, 8 * (tstart + k):8 * (tstart + k + Tc)],
            num_idxs=128 * Tc, num_idxs_reg=128 * Tc, elem_size=elem)
        k += Tc


def _chunks(NPC):
    out = []
    st = 0
    while st < NPC:
        out.append((st, min(128, NPC - st)))
        st += 128
    return out


def build(N, schedule, NBLK, TT, CWF, CWB):
    NPC = N // NCORES
    HALF = N // 2
    nc = bacc.Bacc("TRN2", target_bir_lowering=False, debug=False)

    xT = nc.dram_tensor("xT", [D_IN, NPC], BF16, kind="ExternalInput")
    idx_xl = nc.dram_tensor("idx_xl", [128, 8 * TT], I16, kind="ExternalInput")
    idx_xr = nc.dram_tensor("idx_xr", [128, 8 * TT], I16, kind="ExternalInput")
    dstl = nc.dram_tensor("dstl", [128, TT], F32, kind="ExternalInput")
    constsF = nc.dram_tensor("constsF", [128, CWF], F32, kind="ExternalInput")
    constsB = nc.dram_tensor("constsB", [128, CWB], BF16, kind="ExternalInput")
    out = nc.dram_tensor("out", [NPC, OUT], F32, kind="ExternalOutput")

    ohs_d = nc.dram_tensor("ohs_d", [128, TT * 128], BF16)
    xl0_own = nc.dram_tensor("xl0_own", [NPC, HC], BF16)
    xl0_full = nc.dram_tensor("xl0_full", [N, HC], BF16, addr_space="Shared")
    xr0 = nc.dram_tensor("xr0", [NPC, HC], BF16)
    h1_own = nc.dram_tensor("h1_own", [NPC, HC], BF16)
    h1T = nc.dram_tensor("h1T", [HC, NPC], BF16)
    xl1_own = nc.dram_tensor("xl1_own", [NPC, HC], BF16)
    xl1_full = nc.dram_tensor("xl1_full", [N, HC], BF16, addr_space="Shared")
    xr1 = nc.dram_tensor("xr1", [NPC, HC], BF16)
    h2T = nc.dram_tensor("h2T", [HC, NPC], BF16)
    xl2p8_own = nc.dram_tensor("xl2p8_own", [NPC, 8], BF16)
    xl2p8_full = nc.dram_tensor("xl2p8_full", [N, 8], BF16, addr_space="Shared")
    xl2pB = nc.dram_tensor("xl2pB", [N, 128], BF16)  # cols 0:8 valid
    xr2p = nc.dram_tensor("xr2p", [NPC, 128], BF16)  # cols 0:2 valid

    chunks = _chunks(NPC)
    blk_tiles = {b: [] for b in range(NBLK)}
    t0 = 0
    for b, nm, T in schedule:
        blk_tiles[b].append((t0, nm, T))
        t0 += T
    TMAX = max(sum(T for _, _, T in blk_tiles[b]) for b in range(NBLK))

    rg = [list(range(NCORES))]

    with tile.TileContext(nc) as tc:
        import contextlib
        with contextlib.ExitStack() as ctx:
            cst = ctx.enter_context(tc.tile_pool(name="cst", bufs=1))
            sb = ctx.enter_context(tc.tile_pool(name="sb", bufs=3))
            eb = ctx.enter_context(tc.tile_pool(name="eb", bufs=2))
            eb1 = ctx.enter_context(tc.tile_pool(name="eb1", bufs=1))
            gat = ctx.enter_context(tc.tile_pool(name="gat", bufs=2))
            ohp = ctx.enter_context(tc.tile_pool(name="ohp", bufs=2))
            ps = ctx.enter_context(tc.tile_pool(name="ps", bufs=2, space="PSUM"))
            psa = ctx.enter_context(tc.tile_pool(name="psa", bufs=2, space="PSUM"))

            CF = cst.tile([128, CWF], F32)
            nc.sync.dma_start(CF[:], constsF[:])
            CB = cst.tile([128, CWB], BF16)
            nc.sync.dma_start(CB[:], constsB[:])

            def csF(name):
                r, c0i, w = COLSF[name]
                return CF[0:r, c0i:c0i + w]

            def csB(name):
                r, c0i, w = COLSB[name]
                return CB[0:r, c0i:c0i + w]

            identB = cst.tile([128, 128], BF16)
            nc.vector.tensor_scalar(out=identB[:], in0=csB("iota"),
                                    scalar1=csF("iotaC"), scalar2=None,
                                    op0=ALU.is_equal)
            ixl_t = cst.tile([128, 8 * TT], I16)
            nc.sync.dma_start(ixl_t[:], idx_xl[:])
            ixr_t = cst.tile([128, 8 * TT], I16)
            nc.sync.dma_start(ixr_t[:], idx_xr[:])
            dstl_t = cst.tile([128, TT], F32)
            nc.sync.dma_start(dstl_t[:], dstl[:])
            o_all = cst.tile([128, 2 * NBLK], F32)
            h0T = cst.tile([64, NPC], BF16)

            # prebuild all one-hot tiles into DRAM (reused by all 3 edge passes)
            for k0 in range(0, TT, 8):
                kc = min(8, TT - k0)
                bb = ohp.tile([128, 8, 128], BF16, tag="ohbuild")
                for j in range(kc):
                    nc.vector.tensor_scalar(
                        out=bb[:, j, :], in0=csB("iota"),
                        scalar1=dstl_t[:, k0 + j:k0 + j + 1], scalar2=None,
                        op0=ALU.is_equal)
                nc.sync.dma_start(ohs_d[:, k0 * 128:(k0 + kc) * 128],
                                  bb[:, :kc, :])

            # ---------------- phase A: L0 node prep ----------------
            for st, sz in chunks:
                xTc = sb.tile([D_IN, 128], BF16, tag="xTc")
                nc.sync.dma_start(xTc[:, :sz], xT[:, st:st + sz])
                p1 = psa.tile([64, 128], F32, tag="prep", space="PSUM")
                nc.tensor.matmul(p1[:, :sz], lhsT=csB("W_in"), rhs=xTc[:, :sz],
                                 start=True, stop=True)
                nc.scalar.activation(h0T[:, st:st + sz], p1[:, :sz], AF.Relu,
                                     bias=csF("b_in"))
                for W, tab in (("Wl0", xl0_own), ("Wr0", xr0)):
                    p2 = psa.tile([128, HC], F32, tag="prep", space="PSUM")
                    nc.tensor.matmul(p2[:sz, :], lhsT=h0T[:, st:st + sz],
                                     rhs=csB(W), start=True, stop=True)
                    cp = sb.tile([128, HC], BF16, tag="cpA")
                    nc.scalar.copy(cp[:sz, :], p2[:sz, :])
                    nc.sync.dma_start(tab[st:st + sz, :], cp[:sz, :])

            nc.gpsimd.collective_compute(
                "AllGather", ALU.bypass, ins=[xl0_own[:]], outs=[xl0_full[:]],
                replica_groups=rg)

            # ---------------- edge pass for layers 0/1 ----------------
            def edge_pass(lidx, xl_full, xr_tab, attB, aB, bB, hT_out,
                          h_own_out, residual, prep_fn=None):
                for b in range(NBLK):
                    st = b * BLOCK
                    nreal = min(BLOCK, NPC - st)
                    tl = blk_tiles[b]
                    T_all = sum(T for _, _, T in tl)
                    tglob0 = tl[0][0]
                    # gathers into one block-wide tile
                    gxl = gat.tile([128, TMAX, HC], BF16, tag="gxl")
                    tloc = 0
                    for (tg, nm, T) in tl:
                        src_ap = xl_full[0:HALF, :] if nm == "lo" else xl_full[HALF:N, :]
                        _gather(nc, gxl, src_ap, ixl_t, tg, tloc, T, HC)
                        tloc += T
                    gxr = gat.tile([128, TMAX, HC], BF16, tag="gxr")
                    _gather(nc, gxr, xr_tab[:], ixr_t, tglob0, 0, T_all, HC)
                    if residual is not None:
                        hres = sb.tile([128, HC], BF16, tag="hres")
                        if nreal < 128:
                            nc.vector.memset(hres[:], 0.0)
                        nc.sync.dma_start(hres[:nreal, :], residual[st:st + nreal, :])
                    # block-wide edge math (TT 2x / TS 4x ops)
                    u = eb.tile([128, TMAX, HC], BF16, tag="u")
                    nc.vector.tensor_tensor(
                        out=u[:, :T_all, :], in0=gxl[:, :T_all, :],
                        in1=gxr[:, :T_all, :], op=ALU.add)
                    m = eb.tile([128, TMAX, HC], BF16, tag="m")
                    nc.scalar.activation(m[:, :T_all, :], u[:, :T_all, :],
                                         AF.Prelu, alpha=NEG_SLOPE)
                    p = eb1.tile([128, TMAX, HID, HEADS], BF16, tag="p")
                    nc.vector.tensor_tensor(
                        out=p[:, :T_all, :, :].rearrange("a t c h -> a t (c h)"),
                        in0=m[:, :T_all, :],
                        in1=attB[:, None, :].to_broadcast([128, T_all, HC]),
                        op=ALU.mult)
                    # binary-tree per-head reduction (TT 2x, heads innermost)
                    q1 = eb1.tile([128, TMAX, 32, HEADS], BF16, tag="q1")
                    nc.vector.tensor_tensor(
                        out=q1[:, :T_all], in0=p[:, :T_all, 0:32, :],
                        in1=p[:, :T_all, 32:64, :], op=ALU.add)
                    q2 = eb1.tile([128, TMAX, 16, HEADS], BF16, tag="q2")
                    nc.vector.tensor_tensor(
                        out=q2[:, :T_all], in0=q1[:, :T_all, 0:16, :],
                        in1=q1[:, :T_all, 16:32, :], op=ALU.add)
                    q3 = eb1.tile([128, TMAX, 8, HEADS], BF16, tag="q3")
                    nc.vector.tensor_tensor(
                        out=q3[:, :T_all], in0=q2[:, :T_all, 0:8, :],
                        in1=q2[:, :T_all, 8:16, :], op=ALU.add)
                    q4 = eb1.tile([128, TMAX, 4, HEADS], BF16, tag="q4")
                    nc.vector.tensor_tensor(
                        out=q4[:, :T_all], in0=q3[:, :T_all, 0:4, :],
                        in1=q3[:, :T_all, 4:8, :], op=ALU.add)
                    q5 = eb1.tile([128, TMAX, 2, HEADS], BF16, tag="q5")
                    nc.vector.tensor_tensor(
                        out=q5[:, :T_all], in0=q4[:, :T_all, 0:2, :],
                        in1=q4[:, :T_all, 2:4, :], op=ALU.add)
                    lg = eb.tile([128, TMAX, HEADS], F32, tag="lg")
                    nc.vector.tensor_tensor(
                        out=lg[:, :T_all], in0=q5[:, :T_all, 0, :],
                        in1=q5[:, :T_all, 1, :], op=ALU.add)
                    # combined scatter rhs: [ex-weighted xl | ex] per edge
                    rhsC = eb.tile([128, TMAX, HC + HEADS], BF16, tag="rhsC")
                    ex = rhsC[:, :, HC:HC + HEADS]
                    nc.scalar.activation(ex[:, :T_all], lg[:, :T_all], AF.Exp)
                    # block-wide ex premultiply (heads innermost -> packed 2x)
                    nc.vector.tensor_tensor(
                        out=rhsC[:, :T_all, 0:HC].rearrange("a t (c h) -> a t c h", h=HEADS),
                        in0=gxl[:, :T_all, :].rearrange("a t (c h) -> a t c h", h=HEADS),
                        in1=ex[:, :T_all, None, :].to_broadcast(
                            [128, T_all, HID, HEADS]),
                        op=ALU.mult)
                    # scatter: one matmul per tile (numerator + denominator)
                    acc = ps.tile([128, HC + HEADS], F32, tag="acc", space="PSUM")
                    oh_b = ohp.tile([128, TMAX, 128], BF16, tag="ohb")
                    nc.sync.dma_start(oh_b[:, :T_all, :].rearrange("a t d -> a (t d)"),
                                      ohs_d[:, tglob0 * 128:(tglob0 + T_all) * 128])
                    for t in range(T_all):
                        nc.tensor.matmul(
                            acc[:], lhsT=oh_b[:, t, :], rhs=rhsC[:, t, :],
                            start=(t == 0), stop=(t == T_all - 1))
                    # block post
                    rc = sb.tile([128, HEADS], F32, tag="rc")
                    nc.vector.reciprocal(rc[:], acc[:, HC:HC + HEADS])
                    go = sb.tile([128, HID, HEADS], BF16, tag="go")
                    nc.vector.tensor_tensor(
                        out=go[:], in0=acc[:, 0:HC].rearrange(
                            "a (c h) -> a c h", h=HEADS),
                        in1=rc[:, None, :].to_broadcast([128, HID, HEADS]),
                        op=ALU.mult)
                    t1 = sb.tile([128, HC], BF16, tag="t1")
                    nc.vector.tensor_tensor(
                        out=t1[:], in0=go[:].rearrange("a c h -> a (c h)"),
                        in1=aB, op=ALU.mult)
                    t2 = sb.tile([128, HC], BF16, tag="t2")
                    nc.vector.tensor_tensor(
                        out=t2[:], in0=t1[:], in1=bB, op=ALU.add)
                    h_t = sb.tile([128, HC], BF16, tag="h")
                    if residual is not None:
                        r_t = sb.tile([128, HC], BF16, tag="r")
                        nc.vector.tensor_scalar(
                            out=r_t[:], in0=t2[:], scalar1=0.0, scalar2=None,
                            op0=ALU.max)
                        nc.vector.tensor_tensor(
                            out=h_t[:], in0=r_t[:], in1=hres[:], op=ALU.add)
                    else:
                        nc.vector.tensor_scalar(
                            out=h_t[:], in0=t2[:], scalar1=0.0, scalar2=None,
                            op0=ALU.max)
                    if h_own_out is not None:
                        nc.sync.dma_start(h_own_out[st:st + nreal, :], h_t[:nreal, :])
                    tcps = []
                    for half in range(2):
                        tp = ps.tile([128, 128], BF16, tag="tp", space="PSUM")
                        nc.tensor.transpose(tp[:], h_t[:, half * 128:(half + 1) * 128],
                                            identB[:])
                        tcp = sb.tile([128, 128], BF16, tag=f"tcp{half}")
                        nc.vector.tensor_copy(tcp[:], tp[:])
                        tcps.append(tcp)
                    # fused node prep for the next layer (no DRAM round trip)
                    prep_fn(st, nreal, tcps)

            def prep_l1(st, sz, tcps):
                for Wk0, Wk1, tab in (("Wl1k0", "Wl1k1", xl1_own),
                                      ("Wr1k0", "Wr1k1", xr1)):
                    p2 = psa.tile([128, HC], F32, tag="prep", space="PSUM")
                    nc.tensor.matmul(p2[:sz, :], lhsT=tcps[0][:, :sz], rhs=csB(Wk0),
                                     start=True, stop=False)
                    nc.tensor.matmul(p2[:sz, :], lhsT=tcps[1][:, :sz], rhs=csB(Wk1),
                                     start=False, stop=True)
                    cp = sb.tile([128, HC], BF16, tag="cpA")
                    nc.scalar.copy(cp[:sz, :], p2[:sz, :])
                    nc.sync.dma_start(tab[st:st + sz, :], cp[:sz, :])

            edge_pass(0, xl0_full, xr0, csB("attB0"), csB("aB0"), csB("bB0"),
                      h1T, h1_own, None, prep_l1)

            nc.gpsimd.collective_compute(
                "AllGather", ALU.bypass, ins=[xl1_own[:]], outs=[xl1_full[:]],
                replica_groups=rg)

            def prep_l2(st, sz, tcps):
                p2 = psa.tile([128, OUT], F32, tag="prep", space="PSUM")
                nc.tensor.matmul(p2[:sz, :], lhsT=tcps[0][:, :sz], rhs=csB("Wl2k0"),
                                 start=True, stop=False)
                nc.tensor.matmul(p2[:sz, :], lhsT=tcps[1][:, :sz], rhs=csB("Wl2k1"),
                                 start=False, stop=True)
                cp8 = sb.tile([128, 8], BF16, tag="cp8")
                nc.vector.memset(cp8[:], 0.0)
                nc.scalar.copy(cp8[:sz, 0:OUT], p2[:sz, :])
                nc.sync.dma_start(xl2p8_own[st:st + sz, :], cp8[:sz, :])
                p3 = psa.tile([128, OUT], F32, tag="prep", space="PSUM")
                nc.tensor.matmul(p3[:sz, :], lhsT=tcps[0][:, :sz], rhs=csB("Wr2k0"),
                                 start=True, stop=False)
                nc.tensor.matmul(p3[:sz, :], lhsT=tcps[1][:, :sz], rhs=csB("Wr2k1"),
                                 start=False, stop=True)
                cpr = sb.tile([128, 128], BF16, tag="cpr")
                nc.vector.memset(cpr[:], 0.0)
                nc.scalar.copy(cpr[:sz, 0:OUT], p3[:sz, :])
                nc.sync.dma_start(xr2p[st:st + sz, :], cpr[:sz, :])

            edge_pass(1, xl1_full, xr1, csB("attB1"), csB("aB1"), csB("bB1"),
                      h2T, None, h1_own, prep_l2)

            nc.gpsimd.collective_compute(
                "AllGather", ALU.bypass, ins=[xl2p8_own[:]], outs=[xl2p8_full[:]],
                replica_groups=rg)
            # expand [N, 8] -> cols 0:8 of [N, 128] (cols 8: stay garbage, unused)
            nc.sync.dma_start(xl2pB[:, 0:8], xl2p8_full[:])

            # ---------------- phase F: L2 edge pass ----------------
            att2 = csB("attB2")
            for b in range(NBLK):
                st = b * BLOCK
                nreal = min(BLOCK, NPC - st)
                tl = blk_tiles[b]
                T_all = sum(T for _, _, T in tl)
                tglob0 = tl[0][0]
                gxl = gat.tile([128, TMAX, 128], BF16, tag="gxl")
                tloc = 0
                for (tg, nm, T) in tl:
                    src_ap = xl2pB[0:HALF, :] if nm == "lo" else xl2pB[HALF:N, :]
                    _gather(nc, gxl, src_ap, ixl_t, tg, tloc, T, 128)
                    tloc += T
                gxr = gat.tile([128, TMAX, 128], BF16, tag="gxr")
                _gather(nc, gxr, xr2p[:], ixr_t, tglob0, 0, T_all, 128)
                u2 = eb1.tile([128, TMAX, OUT], BF16, tag="u2")
                nc.vector.scalar_tensor_tensor(
                    out=u2[:, :T_all, :], in0=gxl[:, :T_all, 0:OUT], scalar=1.0,
                    in1=gxr[:, :T_all, 0:OUT], op0=ALU.mult, op1=ALU.add)
                m2 = eb1.tile([128, TMAX, OUT], BF16, tag="m2")
                nc.vector.scalar_tensor_tensor(
                    out=m2[:, :T_all, :], in0=u2[:, :T_all, :], scalar=NEG_SLOPE,
                    in1=u2[:, :T_all, :], op0=ALU.mult, op1=ALU.max)
                p2_ = eb1.tile([128, TMAX, OUT], BF16, tag="p2")
                nc.vector.scalar_tensor_tensor(
                    out=p2_[:, :T_all, :], in0=m2[:, :T_all, :], scalar=1.0,
                    in1=att2[:, None, :].to_broadcast([128, T_all, OUT]),
                    op0=ALU.mult, op1=ALU.mult)
                lg2 = eb.tile([128, TMAX], F32, tag="lg2")
                nc.vector.scalar_tensor_tensor(
                    out=lg2[:, :T_all], in0=p2_[:, :T_all, 0], scalar=1.0,
                    in1=p2_[:, :T_all, 1], op0=ALU.mult, op1=ALU.add)
                rhsC2 = eb.tile([128, TMAX, OUT + 1], BF16, tag="rhsC2")
                ex2 = rhsC2[:, :, OUT:OUT + 1]
                nc.scalar.activation(ex2[:, :T_all, 0], lg2[:, :T_all], AF.Exp)
                nc.vector.tensor_tensor(
                    out=rhsC2[:, :T_all, 0:OUT], in0=gxl[:, :T_all, 0:OUT],
                    in1=ex2[:, :T_all, :].to_broadcast([128, T_all, OUT]),
                    op=ALU.mult)
                acc2 = ps.tile([128, OUT + 1], F32, tag="acc", space="PSUM")
                oh_b = ohp.tile([128, TMAX, 128], BF16, tag="ohb")
                nc.sync.dma_start(oh_b[:, :T_all, :].rearrange("a t d -> a (t d)"),
                                  ohs_d[:, tglob0 * 128:(tglob0 + T_all) * 128])
                for t in range(T_all):
                    nc.tensor.matmul(acc2[:], lhsT=oh_b[:, t, :], rhs=rhsC2[:, t, :],
                                     start=(t == 0), stop=(t == T_all - 1))
                rc2 = sb.tile([128, 1], F32, tag="rc2")
                nc.vector.reciprocal(rc2[:], acc2[:, OUT:OUT + 1])
                o2 = sb.tile([128, OUT], F32, tag="o2")
                nc.vector.tensor_scalar(out=o2[:], in0=acc2[:, 0:OUT],
                                        scalar1=rc2[:], scalar2=None,
                                        op0=ALU.mult)
                nc.vector.tensor_tensor(out=o_all[:, 2 * b:2 * b + 2], in0=o2[:],
                                        in1=csF("bias2B"), op=ALU.add)

            # ---------------- phase G: log_softmax ----------------
            for b in range(NBLK):
                st = b * BLOCK
                nreal = min(BLOCK, NPC - st)
                d = sb.tile([128, 1], F32, tag="d")
                nc.vector.tensor_tensor(out=d[:], in0=o_all[:, 2 * b + 1:2 * b + 2],
                                        in1=o_all[:, 2 * b:2 * b + 1],
                                        op=ALU.subtract)
                e = sb.tile([128, 1], F32, tag="e")
                nc.scalar.activation(e[:], d[:], AF.Exp)
                ep1 = sb.tile([128, 1], F32, tag="ep1")
                nc.vector.tensor_scalar(out=ep1[:], in0=e[:], scalar1=1.0,
                                        scalar2=None, op0=ALU.add)
                l = sb.tile([128, 1], F32, tag="l")
                nc.scalar.activation(l[:], ep1[:], AF.Ln)
                ls = sb.tile([128, 2], F32, tag="ls")
                nc.vector.tensor_scalar(out=ls[:, 0:1], in0=l[:], scalar1=-1.0,
                                        scalar2=None, op0=ALU.mult)
                nc.vector.tensor_tensor(out=ls[:, 1:2], in0=d[:], in1=l[:],
                                        op=ALU.subtract)
                nc.sync.dma_start(out[st:st + nreal, :], ls[:nreal, :])

    nc.compile()
    return nc


COLSF = None
COLSB = None


# ---------------------------------------------------------------- entry
_CACHE = {}
LAST_RESULTS = None
LAST_NC = None


def kernel(**inputs):
    global COLSF, COLSB, LAST_RESULTS, LAST_NC
    x = np.asarray(inputs["x"], np.float32)
    ei = np.asarray(inputs["edge_index"]).astype(np.int64)
    N = x.shape[0]
    NPC = N // NCORES

    idx_xl, idx_xr, dstl, schedule, NBLK = preprocess(ei, N)
    TT = sum(T for _, _, T in schedule)
    cF, COLSF, cB, COLSB = pack_consts(inputs)

    key = (N, TT, NBLK, tuple(schedule))
    if key not in _CACHE:
        _CACHE[key] = build(N, schedule, NBLK, TT, cF.shape[1], cB.shape[1])
    nc = _CACHE[key]
    LAST_NC = nc

    in_maps = []
    for c in range(NCORES):
        sl = slice(c * NPC, (c + 1) * NPC)
        in_maps.append(dict(
            xT=np.ascontiguousarray(x[sl].T).astype(BFNP),
            idx_xl=idx_xl[c], idx_xr=idx_xr[c], dstl=dstl[c],
            constsF=cF, constsB=cB,
        ))
    res = run_bass_kernel_spmd(nc, in_maps, list(range(NCORES)))
    LAST_RESULTS = res
    outs = [res.results[c]["out"] for c in range(NCORES)]
    return np.concatenate(outs, axis=0).astype(np.float32)
